# revision 1
# baseline (speedup 1.0000x reference)
"""Causal self-attention (b=2, n=2048, d=1024, 16 heads) on 8 NeuronCores.

Sharding: core c handles batch b = c // 4 and head group g = c % 4
(heads 4g..4g+3).  qkv weights column-sharded, proj weights row-sharded
(Megatron); each core emits a partial [2048, 1024] proj output and the
host sums the 4 partials per batch (b_proj also added host-side).

On-device layout (per core, all matmuls in float32r):
  xT   [1024, 2048]  x[b] transposed (host prep)
  qT,kT feature-major [128p, pair, 2048] (2 head pairs, 64-dim heads
        stacked on partitions) -> QK^T computed as S^T[k, q] with two
        K=64 matmuls packed in the PE array via base-partition 0/64.
  V     token-major with a fused ones column per head ([V|1]) so the
        AV matmul also produces the softmax denominator (row 64).
  exp   on ACT (scale=1/8 fused), causal mask = multiplicative f32 tile
        built on gpsimd; only lower-triangle blocks computed.
  normalize: reciprocal on DVE + PE ones-broadcast + DVE multiply.

Emission order is tuned so PE never starves: per token-quarter we do
qkv m-tiles, V blocks, the previous quarter's output projection, then
the attention i-loop with QK running 4 blocks ahead of AV.
"""
import sys

sys.path.insert(0, "/opt/trn_rl_repo")

import numpy as np

import concourse.bass as bass  # noqa: F401
import concourse.mybir as mybir
import concourse.tile as tile
from concourse import bacc
from concourse.bass_utils import run_bass_kernel_spmd

F32 = mybir.dt.float32
F32R = mybir.dt.float32r
Exp = mybir.ActivationFunctionType.Exp
Ident = mybir.ActivationFunctionType.Identity

B = 2
N = 2048
D = 1024
NH = 16
HD = 64
NCORES = 8
GROUPS = 4                # head groups (cores per batch)
HPC = NH // GROUPS        # heads per core = 4
PAIRS = HPC // 2          # head pairs per core = 2
QS = 512                  # q_super width
NQS = N // QS             # 4
NB = N // 128             # 16 token blocks
CCH = D // 128            # 8 contraction chunks

_CACHE = {}


def _build():
    nc = bacc.Bacc("TRN2", target_bir_lowering=False, debug=False,
                   num_devices=NCORES)
    xT = nc.dram_tensor("xT", [D, N], F32R, kind="ExternalInput").ap()
    W = nc.dram_tensor("W", [D, 768], F32R, kind="ExternalInput").ap()
    Wp = nc.dram_tensor("Wp", [256, D], F32R, kind="ExternalInput").ap()
    biasqk = nc.dram_tensor("biasqk", [128, 4], F32, kind="ExternalInput").ap()
    vbias = nc.dram_tensor("vbias", [128, 256], F32, kind="ExternalInput").ap()
    ones64D = nc.dram_tensor("ones64D", [1, 64], F32R, kind="ExternalInput").ap()
    y = nc.dram_tensor("y", [N, D], F32, kind="ExternalOutput").ap()

    with tile.TileContext(nc) as tc:
        with (
            tc.tile_pool(name="persist", bufs=1) as pp,
            tc.tile_pool(name="xtq_pool", bufs=2) as xtq_pool,
            tc.tile_pool(name="et_pool", bufs=8) as et_pool,
            tc.tile_pool(name="work", bufs=3) as work,
            tc.tile_pool(name="ysb_pool", bufs=6) as ysb_pool,
            tc.tile_pool(name="mm", bufs=2, space="PSUM") as mm,
            tc.tile_pool(name="spool", bufs=2, space="PSUM") as spool,
            tc.tile_pool(name="opool", bufs=2, space="PSUM") as opool,
        ):
            # ---- persistent tiles ----
            W_sb = pp.tile([128, CCH, 768], F32R)
            Wp_sb = pp.tile([128, 2, D], F32R)
            bqk_sb = pp.tile([128, 4], F32)
            vbias_sb = pp.tile([128, 256], F32)
            ones64 = pp.tile([1, 64], F32R)
            qT = pp.tile([128, PAIRS, N], F32R)
            kT = pp.tile([128, PAIRS, N], F32R)
            onT = pp.tile([128, PAIRS, N], F32R)
            vaug = pp.tile([128, NB, HPC * 65], F32R)
            vaug_h = vaug.rearrange("p b (h c) -> p b h c", c=65)
            masks = pp.tile([128, 4, QS], F32)

            W_r = W.rearrange("(c p) f -> p c f", p=128)
            Wp_r = Wp.rearrange("(c p) f -> p c f", p=128)
            xT_r = xT.rearrange("(c p) n -> p c n", p=128)
            y_r = y.rearrange("(t p) f -> t p f", p=128)

            # causal masks on gpsimd (off the DMA critical path):
            # masks[p, t, q] = 1.0 iff q - p - 128*t >= 0
            nc.gpsimd.memset(masks[:], 1.0)
            for t in range(4):
                nc.gpsimd.affine_select(
                    out=masks[:, t, :],
                    in_=masks[:, t, :],
                    compare_op=mybir.AluOpType.is_ge,
                    fill=0.0,
                    base=-128 * t,
                    pattern=[[1, QS]],
                    channel_multiplier=-1,
                )

            pending_norm = []

            def emit_norm(bc_on_dve=False):
                """normalize deferred (j, hp, osb) entries: overlap the DVE
                reciprocal chain with the next quarter's PE work.
                bc_on_dve: use DVE for the broadcast copy when flushing into
                an ACT-busy (exp-heavy) window."""
                while pending_norm:
                    j, hp, osb = pending_norm.pop(0)
                    if osb[0].space == bass.MemorySpace.PSUM and j < NQS - 1:
                        # deferred drain: copy here so it queues AFTER the
                        # m-tile copybacks on DVE (slot-recycling order)
                        o_ps = osb
                        osb = {}
                        for h in range(2):
                            osb[h] = work.tile([65, QS], F32, tag="osb",
                                               bufs=4, name=f"osbd{j}{hp}{h}")
                            nc.vector.tensor_copy(osb[h][:], o_ps[h][:])
                    for h in range(2):
                        pb = 64 * h
                        recip = work.tile([1, QS], F32R, tag="recip",
                                          name=f"r{j}{hp}{h}")
                        with nc.allow_low_precision("f32r recip for PE bcast"):
                            nc.vector.reciprocal(recip[:], osb[h][64:65, :])
                        bc_ps = mm.tile([64, QS], F32, tag="mm",
                                        name=f"bc{j}{hp}{h}")
                        nc.tensor.matmul(bc_ps[:], ones64[:], recip[:],
                                         start=True, stop=True)
                        bc_sb = work.tile([64, QS], F32, tag="bc_sb",
                                          name=f"bs{j}{hp}{h}")
                        if bc_on_dve:
                            nc.vector.tensor_copy(bc_sb[:], bc_ps[:])
                        else:
                            nc.scalar.copy(bc_sb[:], bc_ps[:])
                        nc.vector.tensor_mul(
                            onT[pb : pb + 64, hp, QS * j : QS * (j + 1)],
                            osb[h][0:64, :],
                            bc_sb[:],
                        )

            def make_proj_units(jj, tail=False):
                """output projection for quarter jj as one closure per
                (block, half) unit, so units can be interleaved into the
                ACT-bound attention i-loop as PE fillers"""
                def unit(blk, nh):
                    def emit():
                        tb = 4 * jj + blk
                        yps = mm.tile([128, QS], F32, tag="mm",
                                      name=f"y{tb}{nh}")
                        for c in range(2):
                            nc.tensor.matmul(
                                yps[:],
                                onT[:, c, 128 * tb : 128 * (tb + 1)],
                                Wp_sb[:, c, QS * nh : QS * (nh + 1)],
                                start=(c == 0),
                                stop=(c == 1),
                            )
                        ysb = ysb_pool.tile([128, QS], F32, tag="ysb",
                                            name=f"ysb{tb}{nh}")
                        # tail: ACT is idle — alternate copy engines
                        if tail and (blk + nh) % 2 == 1:
                            nc.scalar.copy(ysb[:], yps[:])
                        else:
                            nc.vector.tensor_copy(ysb[:], yps[:])
                        nc.sync.dma_start(
                            y_r[tb][:, QS * nh : QS * (nh + 1)], ysb[:]
                        )
                    return emit
                return [unit(blk, nh) for blk in range(4) for nh in range(2)]

            def emit_proj(jj, tail=False):
                for u in make_proj_units(jj, tail):
                    u()

            def fetch_xq(q):
                t0, t1 = QS * q, QS * (q + 1)
                xq = xtq_pool.tile([128, CCH, QS], F32R, tag="xq",
                                   name=f"xq{q}")
                for ci in range(CCH):
                    nc.sync.dma_start(xq[:, ci, :], xT_r[:, ci, t0:t1])
                return xq

            next_xq = None
            for qtr in range(NQS):
                ts, te = QS * qtr, QS * (qtr + 1)
                j = qtr

                # ---- input DMAs, ordered by first consumption ----
                if qtr == 0:
                    xq = xtq_pool.tile([128, CCH, QS], F32R, tag="xq",
                                       name="xq0")
                    for ci in range(CCH):
                        nc.sync.dma_start(W_sb[:, ci, 0:512], W_r[:, ci, 0:512])
                        nc.sync.dma_start(xq[:, ci, :], xT_r[:, ci, ts:te])
                    nc.sync.dma_start(bqk_sb[:], biasqk)
                    # ones columns of [V|1] via DVE (0*x + 1) — avoids a
                    # descriptor-heavy 64-column scatter DMA
                    nc.vector.tensor_scalar(
                        out=vaug_h[:, :, :, 64],
                        in0=W_sb[:, 0, 0:64].rearrange(
                            "p (a b) -> p a b", b=HPC
                        ),
                        scalar1=0.0,
                        scalar2=1.0,
                        op0=mybir.AluOpType.mult,
                        op1=mybir.AluOpType.add,
                    )
                    # v-columns are consumed late (V runs inside hp0's
                    # attention) — keep them off the critical qk prefix
                    for ci in range(CCH):
                        nc.sync.dma_start(W_sb[:, ci, 512:768],
                                          W_r[:, ci, 512:768])
                    nc.sync.dma_start(vbias_sb[:], vbias)
                    next_xq = fetch_xq(1)
                    nc.sync.dma_start(ones64[:], ones64D)
                    for c in range(2):
                        nc.sync.dma_start(Wp_sb[:, c, :], Wp_r[:, c, :])
                else:
                    xq = next_xq
                    if qtr + 1 < NQS:
                        next_xq = fetch_xq(qtr + 1)

                # ---- qkv projection: q/k feature-major m-tiles ----
                # pair-major halves so pair 0's q AND k finish first;
                # chunk-outer so quarter 0 consumes x chunks as they arrive
                for half in ((0, 2), (1, 3)):
                    ps = {
                        m: mm.tile([128, QS], F32, tag="mm", name=f"qk{qtr}{m}")
                        for m in half
                    }
                    for ci in range(CCH):
                        for m in half:
                            nc.tensor.matmul(
                                ps[m][:],
                                W_sb[:, ci, 128 * m : 128 * (m + 1)],
                                xq[:, ci, :],
                                start=(ci == 0),
                                stop=(ci == CCH - 1),
                            )
                    for m in half:
                        dst = qT if m < 2 else kT
                        nc.vector.tensor_scalar_add(
                            dst[:, m % 2, ts:te], ps[m][:], bqk_sb[:, m : m + 1]
                        )

                # ---- V token-major (with bias) into [V|1] slots ----
                # as filler units: V(blk) is only consumed by the diagonal
                # AVs, which sit in hp0's drain — so V can interleave into
                # the ACT-bound i-loop
                def make_v_units(q=qtr, xq_=xq):
                    def unit(blk):
                        def emit():
                            tb = 4 * q + blk
                            vps = mm.tile([128, 256], F32, tag="mm",
                                          name=f"v{q}{blk}")
                            for ci in range(CCH):
                                nc.tensor.matmul(
                                    vps[:],
                                    xq_[:, ci, 128 * blk : 128 * (blk + 1)],
                                    W_sb[:, ci, 512:768],
                                    start=(ci == 0),
                                    stop=(ci == CCH - 1),
                                )
                            nc.vector.tensor_add(
                                vaug_h[:, tb, :, 0:64],
                                vps.rearrange("p (h c) -> p h c", c=64),
                                vbias_sb.rearrange("p (h c) -> p h c", c=64),
                            )
                        return emit
                    return [unit(blk) for blk in range(4)]

                # previous quarter's normalize fills DVE while this quarter's
                # qT/kT copies complete; this quarter's V units and the
                # previous quarter's proj units are spread into the ACT-bound
                # attention i-loop below as PE fillers (V first — it must
                # land before hp0's diagonal AVs in the drain)
                fillers_v = make_v_units()
                fillers_p = []
                if qtr > 0:
                    emit_norm()
                    fillers_p = make_proj_units(qtr - 1)

                # ---- attention for q_super j ----
                n_i = 4 * j + 4
                slots = max(1, 2 * n_i)
                n_fill = len(fillers_v) + len(fillers_p)
                slot = 0
                popped = 0
                for hp in range(PAIRS):
                    # flush hp0's normalize on the last quarter only (no next
                    # quarter to absorb it); mid-kernel it steals ACT/DVE from
                    # the exp pipeline
                    if qtr == NQS - 1:
                        emit_norm(bc_on_dve=True)
                    o_ps = {
                        h: opool.tile([65, QS], F32, tag="o", name=f"o{j}{hp}{h}")
                        for h in range(2)
                    }
                    ets = {}

                    def blk_qs0(t):
                        # f32r matmuls under 256 moving run at 4 cyc/row, so
                        # keep diag blocks >= 256 wide; the extra columns are
                        # zeroed by the mask before AV
                        return 0 if t < 0 else min(128 * t, QS - 256)

                    def emit_qk(i):
                        t = i - 4 * j
                        qs0 = blk_qs0(t)
                        sps = spool.tile([128, 2, QS], F32, tag="s",
                                         name=f"s{j}{hp}{i}")
                        for h in range(2):
                            pb = 64 * h
                            nc.tensor.matmul(
                                sps[:, h, qs0:],
                                kT[pb : pb + 64, hp, 128 * i : 128 * (i + 1)],
                                qT[pb : pb + 64, hp, QS * j + qs0 : QS * (j + 1)],
                                start=True,
                                stop=True,
                            )
                        et = et_pool.tile([128, 2, QS], F32R, tag="et",
                                          name=f"et{j}{hp}{i}")
                        nc.scalar.activation(
                            et[:, :, qs0:], sps[:, :, qs0:], Exp, scale=0.125,
                        )
                        if t >= 0:
                            # cover [qs0, end of triangle]; columns past the
                            # triangle are all-valid
                            mhi = 128 * t + 128
                            nc.vector.tensor_mul(
                                et[:, :, qs0:mhi],
                                et[:, :, qs0:mhi],
                                masks[:, t, qs0:mhi].unsqueeze(1)
                                .broadcast_to([128, 2, mhi - qs0]),
                            )
                        ets[i] = et

                    def emit_av(i):
                        t = i - 4 * j
                        qs0 = blk_qs0(t)
                        et = ets.pop(i)
                        for h in range(2):
                            hh = (2 * hp + h) * 65
                            nc.tensor.matmul(
                                o_ps[h][:, qs0:],
                                vaug[:, i, hh : hh + 65],
                                et[:, h, qs0:],
                                start=(i == 0),
                                stop=(i == n_i - 1),
                            )

                    LOOKAHEAD = 4
                    for i in range(n_i):
                        emit_qk(i)
                        if i >= LOOKAHEAD:
                            emit_av(i - LOOKAHEAD)
                        slot += 1
                        # spread fillers evenly across the quarter's two hp
                        # segments, skipping the first slots where PE is
                        # still dense with QK pipeline-fill
                        off = 0
                        while (fillers_v or fillers_p) and slot > off and \
                                (slot - off) * n_fill >= \
                                (popped + 1) * max(1, slots - off):
                            popped += 1
                            if fillers_v:
                                fillers_v.pop(0)()
                            else:
                                fillers_p.pop(0)()
                    if hp == 0:
                        # diagonal AVs (in the drain) consume this quarter's
                        # V — flush any V units the i-loop didn't absorb
                        while fillers_v:
                            fillers_v.pop(0)()
                    for i in range(max(0, n_i - LOOKAHEAD), n_i):
                        emit_av(i)

                    if hp == PAIRS - 1:
                        # hp1: defer the o drain into the flush (next quarter
                        # or tail) so it queues after the m-tile copybacks;
                        # the last quarter normalizes straight from PSUM
                        pending_norm.append((j, hp, o_ps))
                    else:
                        # hp0: drain o to SBUF now (DVE idle mid-attention)
                        # to free PSUM for hp1
                        osb = {}
                        for h in range(2):
                            osb[h] = work.tile([65, QS], F32, tag="osb",
                                               bufs=4, name=f"osb{j}{hp}{h}")
                            nc.vector.tensor_copy(osb[h][:], o_ps[h][:])
                        pending_norm.append((j, hp, osb))

                # any proj units not absorbed by the i-loop
                for u in fillers_p:
                    u()

            emit_norm()
            emit_proj(NQS - 1, tail=True)

    nc.compile()
    return nc


def _host_prep(x, W_qkv, b_qkv, W_proj, b_proj):
    """Build per-core input maps."""
    x = np.asarray(x, dtype=np.float32)
    W_qkv = np.asarray(W_qkv, dtype=np.float32)
    b_qkv = np.asarray(b_qkv, dtype=np.float32)
    W_proj = np.asarray(W_proj, dtype=np.float32)

    ones64D = np.ones((1, 64), dtype=np.float32)

    xTs = [np.ascontiguousarray(x[b].T) for b in range(B)]

    in_maps = []
    for c in range(NCORES):
        b, g = divmod(c, GROUPS)
        cols = slice(256 * g, 256 * (g + 1))
        Wslice = np.ascontiguousarray(
            np.concatenate(
                [W_qkv[:, cols], W_qkv[:, 1024:2048][:, cols],
                 W_qkv[:, 2048:3072][:, cols]],
                axis=1,
            )
        )
        bq = b_qkv[cols.start : cols.stop]
        bk = b_qkv[1024 + cols.start : 1024 + cols.stop]
        bv = b_qkv[2048 + cols.start : 2048 + cols.stop]
        biasqk = np.ascontiguousarray(
            np.stack([bq[:128], bq[128:], bk[:128], bk[128:]], axis=1)
        )
        vbias = np.ascontiguousarray(np.broadcast_to(bv, (128, 256)))
        Wp_slice = np.ascontiguousarray(W_proj[cols])
        in_maps.append(
            {
                "xT": xTs[b],
                "W": Wslice,
                "Wp": Wp_slice,
                "biasqk": biasqk,
                "vbias": vbias,
                "ones64D": ones64D,
            }
        )
    return in_maps


def _make_runner(nc):
    """Build the PJRT executable once (mirrors bass2jax.run_bass_via_pjrt)
    so repeated kernel() calls skip re-tracing/compile-cache lookups."""
    import jax
    from jax.sharding import Mesh, PartitionSpec
    from jax.experimental.shard_map import shard_map

    from concourse.bass2jax import (
        _bass_exec_p,
        install_neuronx_cc_hook,
        partition_id_tensor,
    )

    install_neuronx_cc_hook()
    partition_name = (
        nc.partition_id_tensor.name if nc.partition_id_tensor else None
    )
    in_names, out_names, out_avals, zero_outs = [], [], [], []
    for alloc in nc.m.functions[0].allocations:
        if not isinstance(alloc, mybir.MemoryLocationSet):
            continue
        name = alloc.memorylocations[0].name
        if alloc.kind == "ExternalInput":
            if name != partition_name:
                in_names.append(name)
        elif alloc.kind == "ExternalOutput":
            out_names.append(name)
            shape = tuple(alloc.tensor_shape)
            dtype = mybir.dt.np(alloc.dtype)
            out_avals.append(jax.core.ShapedArray(shape, dtype))
            zero_outs.append(np.zeros(shape, dtype))
    n_params = len(in_names)
    all_in_names = in_names + out_names
    if partition_name is not None:
        all_in_names = all_in_names + [partition_name]

    def _body(*args):
        operands = list(args)
        if partition_name is not None:
            operands.append(partition_id_tensor())
        return tuple(
            _bass_exec_p.bind(
                *operands,
                out_avals=tuple(out_avals),
                in_names=tuple(all_in_names),
                out_names=tuple(out_names),
                lowering_input_output_aliases=(),
                sim_require_finite=True,
                sim_require_nnan=True,
                nc=nc,
            )
        )

    devices = jax.devices()[:NCORES]
    mesh = Mesh(np.asarray(devices), ("core",))
    in_specs = (PartitionSpec("core"),) * (n_params + len(out_names))
    out_specs = (PartitionSpec("core"),) * len(out_names)
    fn = jax.jit(
        shard_map(_body, mesh=mesh, in_specs=in_specs,
                  out_specs=out_specs, check_rep=False),
        keep_unused=True,
    )
    concat_zeros = [
        np.zeros((NCORES * z.shape[0], *z.shape[1:]), z.dtype)
        for z in zero_outs
    ]

    def run(in_maps):
        concat_in = [
            np.concatenate([np.asarray(m[name]) for m in in_maps], axis=0)
            for name in in_names
        ]
        out_arrs = fn(*concat_in, *concat_zeros)
        return [
            {
                name: np.asarray(out_arrs[i]).reshape(
                    NCORES, *out_avals[i].shape
                )[c]
                for i, name in enumerate(out_names)
            }
            for c in range(NCORES)
        ]

    return run


def kernel(x, W_qkv, b_qkv, W_proj, b_proj):
    if "nc" not in _CACHE:
        _CACHE["nc"] = _build()
        try:
            _CACHE["run"] = _make_runner(_CACHE["nc"])
        except Exception:
            _CACHE["run"] = None
    in_maps = _host_prep(x, W_qkv, b_qkv, W_proj, b_proj)
    results = None
    if _CACHE["run"] is not None:
        try:
            results = _CACHE["run"](in_maps)
        except Exception:
            results = None
    if results is None:
        # fallback: the stock path
        results = run_bass_kernel_spmd(
            _CACHE["nc"], in_maps, core_ids=list(range(NCORES))
        ).results
    out = np.zeros((B, N, D), dtype=np.float32)
    bp = np.asarray(b_proj, dtype=np.float32)
    for b in range(B):
        acc = results[4 * b]["y"].astype(np.float32).copy()
        for g in range(1, GROUPS):
            acc += results[4 * b + g]["y"]
        out[b] = acc + bp
    return out



# revision 44
# speedup vs baseline: 1.2483x; 1.2483x over previous
"""Causal self-attention (b=2, n=2048, d=1024, 16 heads) on 8 NeuronCores.

Sharding: core c handles batch b = c // 4 and head group g = c % 4
(heads 4g..4g+3).  qkv weights column-sharded, proj weights row-sharded
(Megatron); each core emits a partial [2048, 1024] proj output and the
host sums the 4 partials per batch (biases folded in host-side).

Precision plan (tolerance 2e-2; ~1.5e-2 measured on the real inputs):
  qkv/v projections: fp8e4 DoubleRow with residual compensation — x and
      W are split on host into fp8 main + fp8 residual AT THE SAME SCALE
      (x16 / x64), so the three cross terms (aa, ab, ba) accumulate in
      one PSUM group with no extra vector work.  1.5 cyc/row vs 2 for
      bf16, accuracy better than bf16 (1.2e-3 vs 2.4e-3).
  q, k: cast to fp8 (x16) during the qkv copyback; QK^T runs as fp8
      DoubleRow (0.5 cyc/row): per head the 64 contraction dims live as
      [32 partitions x 2 k-tiles].  Scores land x256 in PSUM; the exp
      scale 0.125/256 folds it back.  (~1.46e-2, the error budget.)
  v / et / masks / onT / W_proj: bf16 (1 cyc/row, 2-byte DVE modes).

Scheduling: ONE global software-pipelined stream over all (quarter,
head-pair, k-block) attention tasks.  QK runs AV_LAG slots ahead of AV;
exp/mask sit between on ACT/DVE.  PE fillers — V blocks, the previous
quarter's output projection, the NEXT quarter's qkv m-tiles — are paced
into the stream so PE never waits on the ACT exp feed, and boundary
drains (hp/quarter) are covered by the next segment's QKs.  Softmax
normalization is deferred via a slot-scheduled action queue: reciprocal
(from PSUM row 64) + drain right after a segment's last AV, the PE
ones-broadcast + 2-byte multiply a few slots later.
"""
import sys

sys.path.insert(0, "/opt/trn_rl_repo")

import numpy as np

import concourse.bass as bass  # noqa: F401
import concourse.mybir as mybir
import concourse.tile as tile
from concourse import bacc
from concourse.bass_utils import run_bass_kernel_spmd

F32 = mybir.dt.float32
F32R = mybir.dt.float32r
BF16 = mybir.dt.bfloat16
FP8 = mybir.dt.float8e4
Exp = mybir.ActivationFunctionType.Exp
DR = mybir.MatmulPerfMode.DoubleRow

B = 2
N = 2048
D = 1024
NH = 16
HD = 64
NCORES = 8
GROUPS = 4                # head groups (cores per batch)
HPC = NH // GROUPS        # heads per core = 4
PAIRS = HPC // 2          # head pairs per core = 2
QS = 512                  # q_super width
NQS = N // QS             # 4
NB = N // 128             # 16 token blocks
CCH = D // 128            # 8 contraction chunks
NDR = CCH // 2            # DoubleRow chunk pairs = 4
XS = 16.0                 # fp8 scale for x
WS = 64.0                 # fp8 scale for W_qkv
QSCALE = 16.0             # fp8 scale for q and k
PSCALE = XS * WS          # qkv PSUM arrives x1024
ESCALE = 0.125 / (QSCALE * QSCALE)
AV_LAG = 4

_CACHE = {}


def _build():
    nc = bacc.Bacc("TRN2", target_bir_lowering=False, debug=False,
                   num_devices=NCORES)
    xa_d = nc.dram_tensor("xa", [D, N], FP8, kind="ExternalInput").ap()
    xb_d = nc.dram_tensor("xb", [D, N], FP8, kind="ExternalInput").ap()
    Wa_d = nc.dram_tensor("Wa", [D, 768], FP8, kind="ExternalInput").ap()
    Wb_d = nc.dram_tensor("Wb", [D, 768], FP8, kind="ExternalInput").ap()
    Wp_d = nc.dram_tensor("Wp", [256, D], BF16, kind="ExternalInput").ap()
    biasqk = nc.dram_tensor("biasqk", [128, 4], F32, kind="ExternalInput").ap()
    ones2D = nc.dram_tensor("ones2D", [1, 64], BF16, kind="ExternalInput").ap()
    y = nc.dram_tensor("y", [N, D], BF16, kind="ExternalOutput").ap()

    with tile.TileContext(nc) as tc:
        with (
            tc.tile_pool(name="persist", bufs=1) as pp,
            tc.tile_pool(name="xtq_pool", bufs=3) as xtq_pool,
            tc.tile_pool(name="et_pool", bufs=10) as et_pool,
            tc.tile_pool(name="work", bufs=3) as work,
            tc.tile_pool(name="ysb_pool", bufs=6) as ysb_pool,
            tc.tile_pool(name="mm", bufs=2, space="PSUM") as mm,
            tc.tile_pool(name="spool", bufs=2, space="PSUM") as spool,
            tc.tile_pool(name="opool", bufs=2, space="PSUM") as opool,
        ):
            # ---- persistent tiles ----
            Wa_sb = pp.tile([128, CCH, 768], FP8)
            Wb_sb = pp.tile([128, CCH, 768], FP8)
            Wp_sb = pp.tile([128, 2, D], BF16)
            bqk_sb = pp.tile([128, 4], F32)
            ones2 = pp.tile([1, 64], BF16)
            qT = pp.tile([128, 2, N], FP8)
            kT = pp.tile([128, 2, N], FP8)
            onT = pp.tile([128, PAIRS, N], BF16)
            vaug = pp.tile([128, NB, HPC * 65], BF16)
            vaug_h = vaug.rearrange("p b (h c) -> p b h c", c=65)
            masks = pp.tile([128, 4, QS], BF16)

            Wa_r = Wa_d.rearrange("(c p) f -> p c f", p=128)
            Wb_r = Wb_d.rearrange("(c p) f -> p c f", p=128)
            xa_r = xa_d.rearrange("(c p) n -> p c n", p=128)
            xb_r = xb_d.rearrange("(c p) n -> p c n", p=128)
            Wp_r = Wp_d.rearrange("(c p) f -> p c f", p=128)
            y_r = y.rearrange("(t p) f -> t p f", p=128)

            # causal masks on gpsimd (off the DMA critical path):
            # masks[p, t, q] = 1.0 iff q - p - 128*t >= 0
            nc.gpsimd.memset(masks[:], 1.0)
            for t in range(4):
                nc.gpsimd.affine_select(
                    out=masks[:, t, :],
                    in_=masks[:, t, :],
                    compare_op=mybir.AluOpType.is_ge,
                    fill=0.0,
                    base=-128 * t,
                    pattern=[[1, QS]],
                    channel_multiplier=-1,
                )

            def fetch_xq(q):
                """prefetch a quarter of x (both residual halves) as two
                big DMAs on the gpsimd SWDGE queue (off the qSP/qAct
                critical paths)"""
                t0, t1 = QS * q, QS * (q + 1)
                xqa = xtq_pool.tile([128, CCH, QS], FP8, tag="xa",
                                    name=f"xa{q}")
                xqb = xtq_pool.tile([128, CCH, QS], FP8, tag="xb",
                                    name=f"xb{q}")
                nc.gpsimd.dma_start(xqa[:, :, :], xa_r[:, :, t0:t1])
                nc.gpsimd.dma_start(xqb[:, :, :], xb_r[:, :, t0:t1])
                return (xqa, xqb)

            def dr3(ps, pairs, first, last):
                """residual DoubleRow passes: pairs yields (lhsT, rhs) APs
                ordered so the main (a,a) terms go first — compute can
                start before the residual tensors finish loading"""
                n = len(pairs)
                for pi, (lh, rh) in enumerate(pairs):
                    nc.tensor.matmul(
                        ps, lh, rh,
                        start=(first and pi == 0),
                        stop=(last and pi == n - 1),
                        perf_mode=DR,
                    )

            def make_qkv_units(q, xq):
                """qkv q/k m-tiles for quarter q, one unit per m-tile
                (kept atomic: the PSUM tile's writers/readers must emit
                consecutively for safe pool recycling).  m = 0,1: q
                feature-tiles 0/1; m = 2,3: k tiles (partition 32a+f maps
                head a, feat f / f+32).  Copyback casts to fp8 with x16
                scale (+ prescaled bias)."""
                xqa, xqb = xq
                ts, te = QS * q, QS * (q + 1)

                def unit(m):
                    def emit():
                        ps = mm.tile([128, QS], F32, tag="mm",
                                     name=f"qk{q}{m}")
                        msl = slice(128 * m, 128 * (m + 1))
                        pairs = []
                        for wsb, xsb in ((Wa_sb, xqa), (Wa_sb, xqb),
                                         (Wb_sb, xqa)):
                            for dr in range(NDR):
                                sl = slice(2 * dr, 2 * dr + 2)
                                pairs.append((wsb[:, sl, msl],
                                              xsb[:, sl, :]))
                        dr3(ps[:], pairs, True, True)
                        dst = qT if m < 2 else kT
                        nc.vector.tensor_scalar(
                            out=dst[:, m % 2, ts:te],
                            in0=ps[:],
                            scalar1=QSCALE / PSCALE,
                            scalar2=bqk_sb[:, m : m + 1],
                            op0=mybir.AluOpType.mult,
                            op1=mybir.AluOpType.add,
                        )
                    return emit
                return [unit(m) for m in range(4)]

            def make_v_units(q, xq):
                """V token-major into [V|1] slots (psum x1024 -> /1024)"""
                xqa, xqb = xq

                def unit(blk):
                    def emit():
                        tb = 4 * q + blk
                        vps = mm.tile([128, 256], F32, tag="mm",
                                      name=f"v{q}{blk}")
                        bsl = slice(128 * blk, 128 * (blk + 1))
                        pairs = []
                        for xsb, wsb in ((xqa, Wa_sb), (xqa, Wb_sb),
                                         (xqb, Wa_sb)):
                            for dr in range(NDR):
                                sl = slice(2 * dr, 2 * dr + 2)
                                pairs.append((xsb[:, sl, bsl],
                                              wsb[:, sl, 512:768]))
                        dr3(vps[:], pairs, True, True)
                        nc.vector.tensor_scalar(
                            out=vaug_h[:, tb, :, 0:64],
                            in0=vps.rearrange("p (h c) -> p h c", c=64),
                            scalar1=1.0 / PSCALE,
                            scalar2=None,
                            op0=mybir.AluOpType.mult,
                        )
                    return emit
                return [unit(blk) for blk in range(4)]

            def make_proj_units(jj, tail=False):
                """output projection for quarter jj, one (block, half) unit"""
                def unit(blk, nh):
                    def emit():
                        tb = 4 * jj + blk
                        yps = mm.tile([128, QS], F32, tag="mm",
                                      name=f"y{tb}{nh}")
                        for c in range(2):
                            nc.tensor.matmul(
                                yps[:],
                                onT[:, c, 128 * tb : 128 * (tb + 1)],
                                Wp_sb[:, c, QS * nh : QS * (nh + 1)],
                                start=(c == 0),
                                stop=(c == 1),
                            )
                        ysb = ysb_pool.tile([128, QS], BF16, tag="ysb",
                                            name=f"ysb{tb}{nh}")
                        # tail: ACT is idle — alternate copy engines
                        if tail and (blk + nh) % 2 == 1:
                            nc.scalar.copy(ysb[:], yps[:])
                        else:
                            nc.vector.tensor_copy(ysb[:], yps[:])
                        nc.sync.dma_start(
                            y_r[tb][:, QS * nh : QS * (nh + 1)], ysb[:]
                        )
                    return emit
                return [unit(blk, nh) for blk in range(4) for nh in range(2)]

            # ---- global attention stream state ----
            tasks = [(j, hp, i)
                     for j in range(NQS)
                     for hp in range(PAIRS)
                     for i in range(4 * j + 4)]
            ets = {}
            o_ps_map = {}
            sched = {}           # slot -> [closures]
            fq_v, fq_qkv, fq_proj = [], [], []

            def emit_qk(j, hp, i):
                t = i - 4 * j
                qs0 = 0 if t < 0 else 128 * t
                sps = spool.tile([128, 2, QS], F32, tag="s",
                                 name=f"s{j}{hp}{i}")
                for h in range(2):
                    a = 2 * hp + h
                    nc.tensor.matmul(
                        sps[:, h, qs0:],
                        kT[32 * a : 32 * a + 32, :, 128 * i : 128 * (i + 1)],
                        qT[32 * a : 32 * a + 32, :,
                           QS * j + qs0 : QS * (j + 1)],
                        start=True,
                        stop=True,
                        perf_mode=DR,
                        tile_position=(32 * a, 0),
                    )
                et = et_pool.tile([128, 2, QS], BF16, tag="et",
                                  name=f"et{j}{hp}{i}")
                nc.scalar.activation(
                    et[:, :, qs0:], sps[:, :, qs0:], Exp, scale=ESCALE,
                )
                if t >= 0:
                    mhi = 128 * t + 128
                    nc.vector.tensor_mul(
                        et[:, :, qs0:mhi],
                        et[:, :, qs0:mhi],
                        masks[:, t, qs0:mhi].unsqueeze(1)
                        .broadcast_to([128, 2, mhi - qs0]),
                    )
                ets[(j, hp, i)] = et

            def norm_finish(j, hp, osb, rec2, on_act=False):
                """ones-broadcast + normalize multiply for segment (j, hp);
                one broadcast matmul per head (output column bases 0/64 —
                single-partition writes off base 0 are illegal, so the two
                reciprocals live on the free dim of one 1-partition tile)"""
                bc_ps = mm.tile([128, QS], F32, tag="mm", name=f"bc{j}{hp}")
                for h in range(2):
                    nc.tensor.matmul(bc_ps[64 * h : 64 * h + 64, :],
                                     ones2[:], rec2[:, h, :],
                                     start=True, stop=True)
                bc_sb = work.tile([128, QS], BF16, tag="bc_sb",
                                  name=f"bs{j}{hp}")
                if on_act:
                    nc.scalar.copy(bc_sb[:], bc_ps[:])
                else:
                    nc.vector.tensor_copy(bc_sb[:], bc_ps[:])
                nc.vector.tensor_mul(
                    onT[:, hp, QS * j : QS * (j + 1)], osb[:], bc_sb[:]
                )
                if hp == PAIRS - 1 and j + 1 < NQS:
                    fq_proj.extend(make_proj_units(j))

            # tail staircase state (used by emit_av on the final
            # segment); tiles and proj units are created lazily at first
            # use so the work-pool rings are at their current position
            jt, hpt = NQS - 1, PAIRS - 1
            tail = {}

            def tail_recip(ch):
                if not tail:
                    tail["rec"] = work.tile([1, 2, QS], BF16, tag="rec",
                                            name="rect")
                    tail["osb"] = work.tile([128, QS], BF16, tag="osb",
                                            bufs=4, name="osbt")
                    tail["proj"] = make_proj_units(jt, tail=True)
                o_ps = o_ps_map[(jt, hpt)]
                csl = slice(128 * ch, 128 * (ch + 1))
                for h in range(2):
                    with nc.allow_low_precision("f32r recip"):
                        nc.vector.reciprocal(tail["rec"][:, h, csl],
                                             o_ps[h][64:65, csl])
                    nc.vector.tensor_copy(
                        tail["osb"][64 * h : 64 * h + 64, csl],
                        o_ps[h][0:64, csl])

            def tail_norm_proj(ch):
                csl = slice(128 * ch, 128 * (ch + 1))
                bc_ps = mm.tile([128, 128], F32, tag="mm", name=f"bct{ch}")
                for h in range(2):
                    nc.tensor.matmul(bc_ps[64 * h : 64 * h + 64, :],
                                     ones2[:], tail["rec"][:, h, csl],
                                     start=True, stop=True)
                bc_sb = work.tile([128, 128], BF16, tag="bc_sb",
                                  name=f"bst{ch}")
                if ch % 2 == 0:
                    nc.scalar.copy(bc_sb[:], bc_ps[:])
                else:
                    nc.vector.tensor_copy(bc_sb[:], bc_ps[:])
                nc.vector.tensor_mul(
                    onT[:, hpt, QS * jt + 128 * ch : QS * jt + 128 * (ch + 1)],
                    tail["osb"][:, csl], bc_sb[:],
                )
                # token block 4*jt+ch is now normalized for both pairs
                tail["proj"][2 * ch]()
                tail["proj"][2 * ch + 1]()

            def emit_av(s, j, hp, i):
                t = i - 4 * j
                qs0 = 0 if t < 0 else 128 * t
                n_i = 4 * j + 4
                if hp == 0 and t == 0:
                    # diagonal AVs consume this quarter's V — force-flush
                    while fq_v:
                        fq_v.pop(0)()
                if i == 0:
                    o_ps_map[(j, hp)] = {
                        h: opool.tile([65, QS], F32, tag="o",
                                      name=f"o{j}{hp}{h}")
                        for h in range(2)
                    }
                o_ps = o_ps_map[(j, hp)]
                et = ets.pop((j, hp, i))
                if (j, hp) == (NQS - 1, PAIRS - 1):
                    # final segment: same trimmed matmuls, but with the
                    # group check skipped (bank flags stay cleared) so the
                    # tail staircase can read finalized o columns while
                    # later diagonal blocks still accumulate — block t
                    # only writes columns >= 128t, so chunk t is final
                    # once block n_i-4+t has landed
                    for h in range(2):
                        hh = (2 * hp + h) * 65
                        nc.tensor.matmul(
                            o_ps[h][:, qs0:],
                            vaug[:, i, hh : hh + 65],
                            et[:, h, qs0:],
                            start=(i == 0),
                            stop=(i == n_i - 1),
                            skip_group_check=True,
                        )
                    if t >= 0:
                        # staircase: column chunk t is final — normalize
                        # it (and run the previous chunk's broadcast,
                        # staggered so PE never waits on the reciprocal)
                        tail_recip(t)
                        if t >= 1:
                            tail_norm_proj(t - 1)
                elif False:
                    pass
                else:
                    for h in range(2):
                        hh = (2 * hp + h) * 65
                        nc.tensor.matmul(
                            o_ps[h][:, qs0:],
                            vaug[:, i, hh : hh + 65],
                            et[:, h, qs0:],
                            start=(i == 0),
                            stop=(i == n_i - 1),
                        )
                if i == n_i - 1 and (j, hp) != (NQS - 1, PAIRS - 1):
                    # segment done: reciprocal from PSUM row 64 + numerator
                    # drain now (frees PSUM); broadcast+multiply a few slots
                    # later so PE's bc matmul never waits on the reciprocal
                    rec2 = work.tile([1, 2, QS], BF16, tag="rec",
                                     name=f"rec{j}{hp}")
                    osb = work.tile([128, QS], BF16, tag="osb", bufs=4,
                                    name=f"osb{j}{hp}")
                    for h in range(2):
                        with nc.allow_low_precision("f32r recip"):
                            nc.vector.reciprocal(rec2[:, h, :],
                                                 o_ps[h][64:65, :])
                        nc.vector.tensor_copy(osb[64 * h : 64 * h + 64, :],
                                              o_ps[h][0:64, :])
                    del o_ps_map[(j, hp)]
                    sched.setdefault(s + 3, []).append(
                        lambda: norm_finish(j, hp, osb, rec2))

            # ---- initial DMAs: big consolidated transfers, spread over
            # the three DGE queues (qSP: x main + y out, qAct: weights,
            # gpsimd SWDGE: x residual + prefetches) so descriptor
            # generation never serializes the startup feed ----
            xqa0 = xtq_pool.tile([128, CCH, QS], FP8, tag="xa", name="xa0")
            xqb0 = xtq_pool.tile([128, CCH, QS], FP8, tag="xb", name="xb0")
            # quarter 0's q/k inputs pair-interleaved: the wire is the
            # startup bottleneck, so feed in consumption order
            for dr in range(NDR):
                sl = slice(2 * dr, 2 * dr + 2)
                nc.sync.dma_start(xqa0[:, sl, :], xa_r[:, sl, 0:QS])
                nc.scalar.dma_start(Wa_sb[:, sl, 0:512], Wa_r[:, sl, 0:512])
            nc.gpsimd.dma_start(xqb0[:, :, :], xb_r[:, :, 0:QS])
            nc.scalar.dma_start(Wb_sb[:, :, 0:512], Wb_r[:, :, 0:512])
            nc.sync.dma_start(bqk_sb[:], biasqk)
            nc.sync.dma_start(ones2[:], ones2D)
            # ones columns of [V|1] via DVE (0*x + 1)
            nc.vector.tensor_scalar(
                out=vaug_h[:, :, :, 64],
                in0=Wa_sb[:, 0, 0:64].rearrange("p (a b) -> p a b", b=HPC),
                scalar1=0.0,
                scalar2=1.0,
                op0=mybir.AluOpType.mult,
                op1=mybir.AluOpType.add,
            )
            # v-columns next (consumed by V fillers early in the stream)
            nc.scalar.dma_start(Wa_sb[:, :, 512:768], Wa_r[:, :, 512:768])
            nc.scalar.dma_start(Wb_sb[:, :, 512:768], Wb_r[:, :, 512:768])
            xq_next = fetch_xq(1)
            nc.scalar.dma_start(Wp_sb[:, :, :], Wp_r[:, :, :])

            # quarter 0's qkv runs up front (nothing to overlap with)
            xq_cur = (xqa0, xqb0)
            for u in make_qkv_units(0, xq_cur):
                u()

            # ---- the global stream ----
            q_first_slot = {}
            acc = 0
            for j in range(NQS):
                q_first_slot[j] = acc
                acc += 2 * (4 * j + 4)
            total_slots = acc

            pace = {}

            def run_sched(s):
                for fn in sched.pop(s, ()):
                    fn()

            # proj units have no deadline — defer them toward quarter 3,
            # where the exp feed (ACT) otherwise outpaces PE's work supply
            PROJ_FRAC = {0: 0.0, 1: 0.0, 2: 0.3, 3: 1.0}

            def pop_fillers(s, j):
                start = q_first_slot[j]
                span = 2 * (4 * j + 4)
                frac = (s - start + 1) / span
                # V done by 40% of quarter (diag AVs), qkv by 85% (the
                # copyback must beat the next quarter's first QK)
                for q_, tgt, key in ((fq_v, 0.40, "v"), (fq_qkv, 0.85, "q")):
                    quota = pace[key + "n"]
                    want = int(quota * min(1.0, frac / tgt) + 0.999)
                    while pace[key] < want and q_:
                        q_.pop(0)()
                        pace[key] += 1
                supply = pace["p"] + len(fq_proj)
                want = int(supply * PROJ_FRAC[j] * frac + 0.999)
                if j == NQS - 1:
                    want = min(want, max(0, supply - 4))  # drain reserve
                while pace["p"] < want and fq_proj:
                    fq_proj.pop(0)()
                    pace["p"] += 1

            for s, (j, hp, i) in enumerate(tasks):
                if hp == 0 and i == 0:
                    # quarter boundary: this quarter's qT/kT must be done
                    while fq_qkv:
                        fq_qkv.pop(0)()
                    fq_v.extend(make_v_units(j, xq_cur))
                    if j + 1 < NQS:
                        xq_cur = xq_next
                        fq_qkv.extend(make_qkv_units(j + 1, xq_cur))
                        if j + 2 < NQS:
                            xq_next = fetch_xq(j + 2)
                    pace.update(vn=len(fq_v), qn=len(fq_qkv),
                                v=0, q=0, p=pace.get("p", 0))
                emit_qk(j, hp, i)
                run_sched(s)
                if s >= AV_LAG:
                    emit_av(s, *tasks[s - AV_LAG])
                pop_fillers(s, j)

            # ---- drain ----
            for idx in range(AV_LAG):
                s = total_slots + idx
                emit_av(s, *tasks[s - AV_LAG])
                run_sched(s)
                while fq_v:
                    fq_v.pop(0)()
                if fq_proj:
                    fq_proj.pop(0)()
            for s in sorted(list(sched)):
                run_sched(s)
            while fq_proj:
                fq_proj.pop(0)()
            tail_norm_proj(3)

    nc.compile()
    return nc


def _host_prep(x, W_qkv, b_qkv, W_proj, b_proj):
    """Build per-core input maps."""
    import ml_dtypes

    bf16 = ml_dtypes.bfloat16
    fp8 = ml_dtypes.float8_e4m3
    x = np.asarray(x, dtype=np.float32)
    W_qkv = np.asarray(W_qkv, dtype=np.float32)
    b_qkv = np.asarray(b_qkv, dtype=np.float32)
    W_proj = np.asarray(W_proj, dtype=np.float32)

    ones2D = np.ones((1, 64), dtype=np.float32).astype(bf16)

    def resid8(a, s):
        a8a = (a * s).astype(fp8)
        a8b = (a * s - a8a.astype(np.float32)).astype(fp8)
        return a8a, a8b

    xabs = []
    for b in range(B):
        xa, xb = resid8(np.ascontiguousarray(x[b].T), XS)
        xabs.append((np.ascontiguousarray(xa), np.ascontiguousarray(xb)))

    # q/k column permutation: new col (t*128 + a*32 + f) <- old col
    # (a*64 + t*32 + f) within the core's 256-column slice
    perm = np.zeros(256, dtype=np.int64)
    for t in range(2):
        for a in range(4):
            for f in range(32):
                perm[t * 128 + a * 32 + f] = a * 64 + t * 32 + f

    in_maps = []
    for c in range(NCORES):
        b, g = divmod(c, GROUPS)
        cs = slice(256 * g, 256 * (g + 1))
        Wq = W_qkv[:, cs][:, perm]
        Wk = W_qkv[:, 1024:2048][:, cs][:, perm]
        Wv = W_qkv[:, 2048:3072][:, cs]
        Wfull = np.concatenate([Wq, Wk, Wv], axis=1)
        Wa, Wb = resid8(Wfull, WS)
        bq = b_qkv[cs.start : cs.stop][perm] * QSCALE
        bk = b_qkv[1024 + cs.start : 1024 + cs.stop][perm] * QSCALE
        biasqk = np.ascontiguousarray(
            np.stack([bq[:128], bq[128:], bk[:128], bk[128:]], axis=1)
        ).astype(np.float32)
        Wp_slice = np.ascontiguousarray(W_proj[cs].astype(bf16))
        in_maps.append(
            {
                "xa": xabs[b][0],
                "xb": xabs[b][1],
                "Wa": np.ascontiguousarray(Wa),
                "Wb": np.ascontiguousarray(Wb),
                "Wp": Wp_slice,
                "biasqk": biasqk,
                "ones2D": ones2D,
            }
        )
    return in_maps


def _make_runner(nc):
    """Build the PJRT executable once (mirrors bass2jax.run_bass_via_pjrt)
    so repeated kernel() calls skip re-tracing/compile-cache lookups."""
    import jax
    from jax.sharding import Mesh, PartitionSpec
    from jax.experimental.shard_map import shard_map

    from concourse.bass2jax import (
        _bass_exec_p,
        install_neuronx_cc_hook,
        partition_id_tensor,
    )

    install_neuronx_cc_hook()
    partition_name = (
        nc.partition_id_tensor.name if nc.partition_id_tensor else None
    )
    in_names, out_names, out_avals, zero_outs = [], [], [], []
    for alloc in nc.m.functions[0].allocations:
        if not isinstance(alloc, mybir.MemoryLocationSet):
            continue
        name = alloc.memorylocations[0].name
        if alloc.kind == "ExternalInput":
            if name != partition_name:
                in_names.append(name)
        elif alloc.kind == "ExternalOutput":
            out_names.append(name)
            shape = tuple(alloc.tensor_shape)
            dtype = mybir.dt.np(alloc.dtype)
            out_avals.append(jax.core.ShapedArray(shape, dtype))
            zero_outs.append(np.zeros(shape, dtype))
    n_params = len(in_names)
    all_in_names = in_names + out_names
    if partition_name is not None:
        all_in_names = all_in_names + [partition_name]

    def _body(*args):
        operands = list(args)
        if partition_name is not None:
            operands.append(partition_id_tensor())
        return tuple(
            _bass_exec_p.bind(
                *operands,
                out_avals=tuple(out_avals),
                in_names=tuple(all_in_names),
                out_names=tuple(out_names),
                lowering_input_output_aliases=(),
                sim_require_finite=True,
                sim_require_nnan=True,
                nc=nc,
            )
        )

    devices = jax.devices()[:NCORES]
    mesh = Mesh(np.asarray(devices), ("core",))
    in_specs = (PartitionSpec("core"),) * (n_params + len(out_names))
    out_specs = (PartitionSpec("core"),) * len(out_names)
    fn = jax.jit(
        shard_map(_body, mesh=mesh, in_specs=in_specs,
                  out_specs=out_specs, check_rep=False),
        keep_unused=True,
    )
    concat_zeros = [
        np.zeros((NCORES * z.shape[0], *z.shape[1:]), z.dtype)
        for z in zero_outs
    ]

    def run(in_maps):
        concat_in = [
            np.concatenate([np.asarray(m[name]) for m in in_maps], axis=0)
            for name in in_names
        ]
        out_arrs = fn(*concat_in, *concat_zeros)
        return [
            {
                name: np.asarray(out_arrs[i]).reshape(
                    NCORES, *out_avals[i].shape
                )[c]
                for i, name in enumerate(out_names)
            }
            for c in range(NCORES)
        ]

    return run


def kernel(x, W_qkv, b_qkv, W_proj, b_proj):
    if "nc" not in _CACHE:
        _CACHE["nc"] = _build()
        try:
            _CACHE["run"] = _make_runner(_CACHE["nc"])
        except Exception:
            _CACHE["run"] = None
    in_maps = _host_prep(x, W_qkv, b_qkv, W_proj, b_proj)
    results = None
    if _CACHE["run"] is not None:
        try:
            results = _CACHE["run"](in_maps)
        except Exception:
            results = None
    if results is None:
        # fallback: the stock path
        results = run_bass_kernel_spmd(
            _CACHE["nc"], in_maps, core_ids=list(range(NCORES))
        ).results
    out = np.zeros((B, N, D), dtype=np.float32)
    bp = np.asarray(b_proj, dtype=np.float32)
    # v-bias contributes bv @ W_proj to every output row (softmax weights
    # sum to 1), so it folds into the projection bias host-side
    bp = bp + np.asarray(b_qkv, np.float32)[2048:] @ np.asarray(
        W_proj, np.float32)
    for b in range(B):
        acc = results[4 * b]["y"].astype(np.float32).copy()
        for g in range(1, GROUPS):
            acc += results[4 * b + g]["y"]
        out[b] = acc + bp
    return out


# revision 62
# speedup vs baseline: 1.2541x; 1.0046x over previous
"""Causal self-attention (b=2, n=2048, d=1024, 16 heads) on 8 NeuronCores.

Sharding: core c handles batch b = c // 4 and head group g = c % 4
(heads 4g..4g+3).  qkv weights column-sharded, proj weights row-sharded
(Megatron); each core emits a partial [2048, 1024] proj output and the
host sums the 4 partials per batch (biases folded in host-side).

Precision plan (tolerance 2e-2; ~1.5e-2 measured on the real inputs):
  qkv/v projections: fp8e4 DoubleRow with residual compensation — x and
      W are split on host into fp8 main + fp8 residual AT THE SAME SCALE
      (x16 / x64), so the three cross terms (aa, ab, ba) accumulate in
      one PSUM group with no extra vector work.  1.5 cyc/row vs 2 for
      bf16, accuracy better than bf16 (1.2e-3 vs 2.4e-3).
  q, k: cast to fp8 (x16) during the qkv copyback; QK^T runs as fp8
      DoubleRow (0.5 cyc/row): per head the 64 contraction dims live as
      [32 partitions x 2 k-tiles].  Scores land x256 in PSUM; the exp
      scale 0.125/256 folds it back.  (~1.46e-2, the error budget.)
  v / et / masks / onT / W_proj: bf16 (1 cyc/row, 2-byte DVE modes).

Scheduling: ONE global software-pipelined stream over all (quarter,
head-pair, k-block) attention tasks.  QK runs AV_LAG slots ahead of AV;
exp/mask sit between on ACT/DVE.  PE fillers — V blocks, the previous
quarter's output projection, the NEXT quarter's qkv m-tiles — are paced
into the stream so PE never waits on the ACT exp feed, and boundary
drains (hp/quarter) are covered by the next segment's QKs.  Softmax
normalization is deferred via a slot-scheduled action queue: reciprocal
(from PSUM row 64) + drain right after a segment's last AV, the PE
ones-broadcast + 2-byte multiply a few slots later.
"""
import sys

sys.path.insert(0, "/opt/trn_rl_repo")

import numpy as np

import concourse.bass as bass  # noqa: F401
import concourse.mybir as mybir
import concourse.tile as tile
from concourse import bacc
from concourse.bass_utils import run_bass_kernel_spmd

F32 = mybir.dt.float32
F32R = mybir.dt.float32r
BF16 = mybir.dt.bfloat16
FP8 = mybir.dt.float8e4
Exp = mybir.ActivationFunctionType.Exp
DR = mybir.MatmulPerfMode.DoubleRow

B = 2
N = 2048
D = 1024
NH = 16
HD = 64
NCORES = 8
GROUPS = 4                # head groups (cores per batch)
HPC = NH // GROUPS        # heads per core = 4
PAIRS = HPC // 2          # head pairs per core = 2
QS = 512                  # q_super width
NQS = N // QS             # 4
NB = N // 128             # 16 token blocks
CCH = D // 128            # 8 contraction chunks
NDR = CCH // 2            # DoubleRow chunk pairs = 4
XS = 16.0                 # fp8 scale for x
WS = 64.0                 # fp8 scale for W_qkv
QSCALE = 16.0             # fp8 scale for q and k
PSCALE = XS * WS          # qkv PSUM arrives x1024
ESCALE = 0.125 / (QSCALE * QSCALE)
AV_LAG = 4

_CACHE = {}


def _build():
    nc = bacc.Bacc("TRN2", target_bir_lowering=False, debug=False,
                   num_devices=NCORES)
    xa_d = nc.dram_tensor("xa", [D, N], FP8, kind="ExternalInput").ap()
    xb_d = nc.dram_tensor("xb", [D, N], FP8, kind="ExternalInput").ap()
    Wa_d = nc.dram_tensor("Wa", [D, 768], FP8, kind="ExternalInput").ap()
    Wb_d = nc.dram_tensor("Wb", [D, 768], FP8, kind="ExternalInput").ap()
    Wp_d = nc.dram_tensor("Wp", [256, D], BF16, kind="ExternalInput").ap()
    biasqk = nc.dram_tensor("biasqk", [128, 4], F32, kind="ExternalInput").ap()
    ones2D = nc.dram_tensor("ones2D", [1, 64], BF16, kind="ExternalInput").ap()
    y = nc.dram_tensor("y", [N, D], BF16, kind="ExternalOutput").ap()

    with tile.TileContext(nc) as tc:
        with (
            tc.tile_pool(name="persist", bufs=1) as pp,
            tc.tile_pool(name="xtq_pool", bufs=3) as xtq_pool,
            tc.tile_pool(name="et_pool", bufs=10) as et_pool,
            tc.tile_pool(name="work", bufs=3) as work,
            tc.tile_pool(name="ysb_pool", bufs=6) as ysb_pool,
            tc.tile_pool(name="mm", bufs=2, space="PSUM") as mm,
            tc.tile_pool(name="spool", bufs=2, space="PSUM") as spool,
            tc.tile_pool(name="opool", bufs=2, space="PSUM") as opool,
        ):
            # ---- persistent tiles ----
            Wa_sb = pp.tile([128, CCH, 768], FP8)
            Wb_sb = pp.tile([128, CCH, 768], FP8)
            Wp_sb = pp.tile([128, 2, D], BF16)
            bqk_sb = pp.tile([128, 4], F32)
            ones2 = pp.tile([1, 64], BF16)
            qT = pp.tile([128, 2, N], FP8)
            kT = pp.tile([128, 2, N], FP8)
            onT = pp.tile([128, PAIRS, N], BF16)
            vaug = pp.tile([128, NB, HPC * 65], BF16)
            vaug_h = vaug.rearrange("p b (h c) -> p b h c", c=65)
            masks = pp.tile([128, 4, QS], BF16)

            Wa_r = Wa_d.rearrange("(c p) f -> p c f", p=128)
            Wb_r = Wb_d.rearrange("(c p) f -> p c f", p=128)
            xa_r = xa_d.rearrange("(c p) n -> p c n", p=128)
            xb_r = xb_d.rearrange("(c p) n -> p c n", p=128)
            Wp_r = Wp_d.rearrange("(c p) f -> p c f", p=128)
            y_r = y.rearrange("(t p) f -> t p f", p=128)

            # causal masks on gpsimd (off the DMA critical path):
            # masks[p, t, q] = 1.0 iff q - p - 128*t >= 0
            nc.gpsimd.memset(masks[:], 1.0)
            for t in range(4):
                nc.gpsimd.affine_select(
                    out=masks[:, t, :],
                    in_=masks[:, t, :],
                    compare_op=mybir.AluOpType.is_ge,
                    fill=0.0,
                    base=-128 * t,
                    pattern=[[1, QS]],
                    channel_multiplier=-1,
                )

            def fetch_xq(q):
                """prefetch a quarter of x (both residual halves) as two
                big DMAs on the gpsimd SWDGE queue (off the qSP/qAct
                critical paths)"""
                t0, t1 = QS * q, QS * (q + 1)
                xqa = xtq_pool.tile([128, CCH, QS], FP8, tag="xa",
                                    name=f"xa{q}")
                xqb = xtq_pool.tile([128, CCH, QS], FP8, tag="xb",
                                    name=f"xb{q}")
                nc.gpsimd.dma_start(xqa[:, :, :], xa_r[:, :, t0:t1])
                nc.gpsimd.dma_start(xqb[:, :, :], xb_r[:, :, t0:t1])
                return (xqa, xqb)

            def dr3(ps, pairs, first, last):
                """residual DoubleRow passes: pairs yields (lhsT, rhs) APs
                ordered so the main (a,a) terms go first — compute can
                start before the residual tensors finish loading"""
                n = len(pairs)
                for pi, (lh, rh) in enumerate(pairs):
                    nc.tensor.matmul(
                        ps, lh, rh,
                        start=(first and pi == 0),
                        stop=(last and pi == n - 1),
                        perf_mode=DR,
                    )

            def make_qkv_units(q, xq):
                """qkv q/k m-tiles for quarter q, one unit per m-tile
                (kept atomic: the PSUM tile's writers/readers must emit
                consecutively for safe pool recycling).  m = 0,1: q
                feature-tiles 0/1; m = 2,3: k tiles (partition 32a+f maps
                head a, feat f / f+32).  Copyback casts to fp8 with x16
                scale (+ prescaled bias)."""
                xqa, xqb = xq
                ts, te = QS * q, QS * (q + 1)

                def unit(m):
                    def emit():
                        ps = mm.tile([128, QS], F32, tag="mm",
                                     name=f"qk{q}{m}")
                        msl = slice(128 * m, 128 * (m + 1))
                        pairs = []
                        for wsb, xsb in ((Wa_sb, xqa), (Wb_sb, xqa),
                                         (Wa_sb, xqb)):
                            for dr in range(NDR):
                                sl = slice(2 * dr, 2 * dr + 2)
                                pairs.append((wsb[:, sl, msl],
                                              xsb[:, sl, :]))
                        dr3(ps[:], pairs, True, True)
                        dst = qT if m < 2 else kT
                        nc.vector.tensor_scalar(
                            out=dst[:, m % 2, ts:te],
                            in0=ps[:],
                            scalar1=QSCALE / PSCALE,
                            scalar2=bqk_sb[:, m : m + 1],
                            op0=mybir.AluOpType.mult,
                            op1=mybir.AluOpType.add,
                        )
                    return emit
                return [unit(m) for m in range(4)]

            def make_v_units(q, xq):
                """V token-major into [V|1] slots (psum x1024 -> /1024)"""
                xqa, xqb = xq

                def unit(blk):
                    def emit():
                        tb = 4 * q + blk
                        vps = mm.tile([128, 256], F32, tag="mm",
                                      name=f"v{q}{blk}")
                        bsl = slice(128 * blk, 128 * (blk + 1))
                        pairs = []
                        for xsb, wsb in ((xqa, Wa_sb), (xqa, Wb_sb),
                                         (xqb, Wa_sb)):
                            for dr in range(NDR):
                                sl = slice(2 * dr, 2 * dr + 2)
                                pairs.append((xsb[:, sl, bsl],
                                              wsb[:, sl, 512:768]))
                        dr3(vps[:], pairs, True, True)
                        nc.vector.tensor_scalar(
                            out=vaug_h[:, tb, :, 0:64],
                            in0=vps.rearrange("p (h c) -> p h c", c=64),
                            scalar1=1.0 / PSCALE,
                            scalar2=None,
                            op0=mybir.AluOpType.mult,
                        )
                    return emit
                return [unit(blk) for blk in range(4)]

            def make_proj_units(jj, tail=False):
                """output projection for quarter jj, one (block, half) unit"""
                def unit(blk, nh):
                    def emit():
                        tb = 4 * jj + blk
                        yps = mm.tile([128, QS], F32, tag="mm",
                                      name=f"y{tb}{nh}")
                        for c in range(2):
                            nc.tensor.matmul(
                                yps[:],
                                onT[:, c, 128 * tb : 128 * (tb + 1)],
                                Wp_sb[:, c, QS * nh : QS * (nh + 1)],
                                start=(c == 0),
                                stop=(c == 1),
                            )
                        ysb = ysb_pool.tile([128, QS], BF16, tag="ysb",
                                            name=f"ysb{tb}{nh}")
                        # tail: ACT is idle — alternate copy engines
                        if tail and (blk + nh) % 2 == 1:
                            nc.scalar.copy(ysb[:], yps[:])
                        else:
                            nc.vector.tensor_copy(ysb[:], yps[:])
                        nc.sync.dma_start(
                            y_r[tb][:, QS * nh : QS * (nh + 1)], ysb[:]
                        )
                    return emit
                return [unit(blk, nh) for blk in range(4) for nh in range(2)]

            # ---- global attention stream state ----
            tasks = [(j, hp, i)
                     for j in range(NQS)
                     for hp in range(PAIRS)
                     for i in range(4 * j + 4)]
            ets = {}
            o_ps_map = {}
            sched = {}           # slot -> [closures]
            fq_v, fq_qkv, fq_proj = [], [], []

            def emit_qk(j, hp, i):
                t = i - 4 * j
                qs0 = 0 if t < 0 else 128 * t
                sps = spool.tile([128, 2, QS], F32, tag="s",
                                 name=f"s{j}{hp}{i}")
                for h in range(2):
                    a = 2 * hp + h
                    nc.tensor.matmul(
                        sps[:, h, qs0:],
                        kT[32 * a : 32 * a + 32, :, 128 * i : 128 * (i + 1)],
                        qT[32 * a : 32 * a + 32, :,
                           QS * j + qs0 : QS * (j + 1)],
                        start=True,
                        stop=True,
                        perf_mode=DR,
                        tile_position=(32 * a, 0),
                    )
                et = et_pool.tile([128, 2, QS], BF16, tag="et",
                                  name=f"et{j}{hp}{i}")
                nc.scalar.activation(
                    et[:, :, qs0:], sps[:, :, qs0:], Exp, scale=ESCALE,
                )
                if t >= 0:
                    mhi = 128 * t + 128
                    nc.vector.tensor_mul(
                        et[:, :, qs0:mhi],
                        et[:, :, qs0:mhi],
                        masks[:, t, qs0:mhi].unsqueeze(1)
                        .broadcast_to([128, 2, mhi - qs0]),
                    )
                ets[(j, hp, i)] = et

            def norm_finish(j, hp, osb, rec2, on_act=False):
                """ones-broadcast + normalize multiply for segment (j, hp);
                one broadcast matmul per head (output column bases 0/64 —
                single-partition writes off base 0 are illegal, so the two
                reciprocals live on the free dim of one 1-partition tile)"""
                bc_ps = mm.tile([128, QS], F32, tag="mm", name=f"bc{j}{hp}")
                for h in range(2):
                    nc.tensor.matmul(bc_ps[64 * h : 64 * h + 64, :],
                                     ones2[:], rec2[:, h, :],
                                     start=True, stop=True)
                bc_sb = work.tile([128, QS], BF16, tag="bc_sb",
                                  name=f"bs{j}{hp}")
                if on_act:
                    nc.scalar.copy(bc_sb[:], bc_ps[:])
                else:
                    nc.vector.tensor_copy(bc_sb[:], bc_ps[:])
                nc.vector.tensor_mul(
                    onT[:, hp, QS * j : QS * (j + 1)], osb[:], bc_sb[:]
                )
                if hp == PAIRS - 1 and j + 1 < NQS:
                    fq_proj.extend(make_proj_units(j))

            # tail staircase state (used by emit_av on the final
            # segment); tiles and proj units are created lazily at first
            # use so the work-pool rings are at their current position
            jt, hpt = NQS - 1, PAIRS - 1
            tail = {}

            def tail_recip(ch):
                if not tail:
                    tail["rec"] = work.tile([1, 2, QS], BF16, tag="rec",
                                            name="rect")
                    tail["osb"] = work.tile([128, QS], BF16, tag="osb",
                                            bufs=4, name="osbt")
                    tail["proj"] = make_proj_units(jt, tail=True)
                o_ps = o_ps_map[(jt, hpt)]
                csl = slice(128 * ch, 128 * (ch + 1))
                for h in range(2):
                    with nc.allow_low_precision("f32r recip"):
                        nc.vector.reciprocal(tail["rec"][:, h, csl],
                                             o_ps[h][64:65, csl])
                    nc.vector.tensor_copy(
                        tail["osb"][64 * h : 64 * h + 64, csl],
                        o_ps[h][0:64, csl])

            def tail_norm_proj(ch):
                csl = slice(128 * ch, 128 * (ch + 1))
                bc_ps = mm.tile([128, 128], F32, tag="mm", name=f"bct{ch}")
                for h in range(2):
                    nc.tensor.matmul(bc_ps[64 * h : 64 * h + 64, :],
                                     ones2[:], tail["rec"][:, h, csl],
                                     start=True, stop=True)
                bc_sb = work.tile([128, 128], BF16, tag="bc_sb",
                                  name=f"bst{ch}")
                if ch % 2 == 0:
                    nc.scalar.copy(bc_sb[:], bc_ps[:])
                else:
                    nc.vector.tensor_copy(bc_sb[:], bc_ps[:])
                nc.vector.tensor_mul(
                    onT[:, hpt, QS * jt + 128 * ch : QS * jt + 128 * (ch + 1)],
                    tail["osb"][:, csl], bc_sb[:],
                )
                # token block 4*jt+ch is now normalized for both pairs
                tail["proj"][2 * ch]()
                tail["proj"][2 * ch + 1]()

            def emit_av(s, j, hp, i):
                t = i - 4 * j
                qs0 = 0 if t < 0 else 128 * t
                n_i = 4 * j + 4
                if hp == 0 and t == 0:
                    # diagonal AVs consume this quarter's V — force-flush
                    while fq_v:
                        fq_v.pop(0)()
                if i == 0:
                    o_ps_map[(j, hp)] = {
                        h: opool.tile([65, QS], F32, tag="o",
                                      name=f"o{j}{hp}{h}")
                        for h in range(2)
                    }
                o_ps = o_ps_map[(j, hp)]
                et = ets.pop((j, hp, i))
                if (j, hp) == (NQS - 1, PAIRS - 1):
                    # final segment: same trimmed matmuls, but with the
                    # group check skipped (bank flags stay cleared) so the
                    # tail staircase can read finalized o columns while
                    # later diagonal blocks still accumulate — block t
                    # only writes columns >= 128t, so chunk t is final
                    # once block n_i-4+t has landed
                    for h in range(2):
                        hh = (2 * hp + h) * 65
                        nc.tensor.matmul(
                            o_ps[h][:, qs0:],
                            vaug[:, i, hh : hh + 65],
                            et[:, h, qs0:],
                            start=(i == 0),
                            stop=(i == n_i - 1),
                            skip_group_check=True,
                        )
                    if t >= 0:
                        # staircase: column chunk t is final — normalize
                        # it (and run the previous chunk's broadcast,
                        # staggered so PE never waits on the reciprocal)
                        tail_recip(t)
                        if t >= 1:
                            tail_norm_proj(t - 1)
                elif False:
                    pass
                else:
                    for h in range(2):
                        hh = (2 * hp + h) * 65
                        nc.tensor.matmul(
                            o_ps[h][:, qs0:],
                            vaug[:, i, hh : hh + 65],
                            et[:, h, qs0:],
                            start=(i == 0),
                            stop=(i == n_i - 1),
                        )
                if i == n_i - 1 and (j, hp) != (NQS - 1, PAIRS - 1):
                    # segment done: reciprocal from PSUM row 64 + numerator
                    # drain now (frees PSUM); broadcast+multiply a few slots
                    # later so PE's bc matmul never waits on the reciprocal
                    rec2 = work.tile([1, 2, QS], BF16, tag="rec",
                                     name=f"rec{j}{hp}")
                    osb = work.tile([128, QS], BF16, tag="osb", bufs=4,
                                    name=f"osb{j}{hp}")
                    for h in range(2):
                        with nc.allow_low_precision("f32r recip"):
                            nc.vector.reciprocal(rec2[:, h, :],
                                                 o_ps[h][64:65, :])
                        nc.vector.tensor_copy(osb[64 * h : 64 * h + 64, :],
                                              o_ps[h][0:64, :])
                    del o_ps_map[(j, hp)]
                    sched.setdefault(s + 3, []).append(
                        lambda: norm_finish(j, hp, osb, rec2))

            # ---- initial DMAs: big consolidated transfers, spread over
            # the three DGE queues (qSP: x main + y out, qAct: weights,
            # gpsimd SWDGE: x residual + prefetches) so descriptor
            # generation never serializes the startup feed ----
            xqa0 = xtq_pool.tile([128, CCH, QS], FP8, tag="xa", name="xa0")
            xqb0 = xtq_pool.tile([128, CCH, QS], FP8, tag="xb", name="xb0")
            # quarter 0's q/k inputs pair-interleaved: the wire is the
            # startup bottleneck, so feed in consumption order
            for dr in range(NDR):
                sl = slice(2 * dr, 2 * dr + 2)
                nc.sync.dma_start(xqa0[:, sl, :], xa_r[:, sl, 0:QS])
                nc.scalar.dma_start(Wa_sb[:, sl, 0:512], Wa_r[:, sl, 0:512])
            nc.gpsimd.dma_start(xqb0[:, :, :], xb_r[:, :, 0:QS])
            nc.scalar.dma_start(Wb_sb[:, :, 0:512], Wb_r[:, :, 0:512])
            nc.sync.dma_start(bqk_sb[:], biasqk)
            nc.sync.dma_start(ones2[:], ones2D)
            # ones columns of [V|1] via DVE (0*x + 1)
            nc.vector.tensor_scalar(
                out=vaug_h[:, :, :, 64],
                in0=Wa_sb[:, 0, 0:64].rearrange("p (a b) -> p a b", b=HPC),
                scalar1=0.0,
                scalar2=1.0,
                op0=mybir.AluOpType.mult,
                op1=mybir.AluOpType.add,
            )
            # v-columns next (consumed by V fillers early in the stream)
            nc.scalar.dma_start(Wa_sb[:, :, 512:768], Wa_r[:, :, 512:768])
            nc.scalar.dma_start(Wb_sb[:, :, 512:768], Wb_r[:, :, 512:768])
            xq_next = fetch_xq(1)
            nc.scalar.dma_start(Wp_sb[:, :, :], Wp_r[:, :, :])

            # quarter 0's qkv runs up front (nothing to overlap with)
            xq_cur = (xqa0, xqb0)
            for u in make_qkv_units(0, xq_cur):
                u()

            # ---- the global stream ----
            q_first_slot = {}
            acc = 0
            for j in range(NQS):
                q_first_slot[j] = acc
                acc += 2 * (4 * j + 4)
            total_slots = acc

            pace = {}

            def run_sched(s):
                for fn in sched.pop(s, ()):
                    fn()

            # proj units have no deadline — defer them toward quarter 3,
            # where the exp feed (ACT) otherwise outpaces PE's work supply
            PROJ_FRAC = {0: 0.0, 1: 0.0, 2: 0.15, 3: 1.0}

            def pop_fillers(s, j):
                start = q_first_slot[j]
                span = 2 * (4 * j + 4)
                frac = (s - start + 1) / span
                # V done by 40% of quarter (diag AVs), qkv by 85% (the
                # copyback must beat the next quarter's first QK)
                for q_, tgt, key in ((fq_v, 0.40, "v"), (fq_qkv, 0.85, "q")):
                    quota = pace[key + "n"]
                    want = int(quota * min(1.0, frac / tgt) + 0.999)
                    while pace[key] < want and q_:
                        q_.pop(0)()
                        pace[key] += 1
                supply = pace["p"] + len(fq_proj)
                want = int(supply * PROJ_FRAC[j] * frac + 0.999)
                if j == NQS - 1:
                    want = min(want, max(0, supply - 4))  # drain reserve
                while pace["p"] < want and fq_proj:
                    fq_proj.pop(0)()
                    pace["p"] += 1

            for s, (j, hp, i) in enumerate(tasks):
                if hp == 0 and i == 0:
                    # quarter boundary: this quarter's qT/kT must be done
                    while fq_qkv:
                        fq_qkv.pop(0)()
                    fq_v.extend(make_v_units(j, xq_cur))
                    if j + 1 < NQS:
                        xq_cur = xq_next
                        fq_qkv.extend(make_qkv_units(j + 1, xq_cur))
                        if j + 2 < NQS:
                            xq_next = fetch_xq(j + 2)
                    pace.update(vn=len(fq_v), qn=len(fq_qkv),
                                v=0, q=0, p=pace.get("p", 0))
                    if j > 0 and fq_v:
                        # one V filler between the qkv flush and the first
                        # QK so DVE can finish the qT/kT copybacks
                        fq_v.pop(0)()
                        pace["v"] += 1
                emit_qk(j, hp, i)
                run_sched(s)
                if s >= AV_LAG:
                    emit_av(s, *tasks[s - AV_LAG])
                pop_fillers(s, j)

            # ---- drain ----
            for idx in range(AV_LAG):
                s = total_slots + idx
                emit_av(s, *tasks[s - AV_LAG])
                run_sched(s)
                while fq_v:
                    fq_v.pop(0)()
                if fq_proj:
                    fq_proj.pop(0)()
            for s in sorted(list(sched)):
                run_sched(s)
            while fq_proj:
                fq_proj.pop(0)()
            tail_norm_proj(3)

    nc.compile()
    return nc


def _host_prep(x, W_qkv, b_qkv, W_proj, b_proj):
    """Build per-core input maps."""
    import ml_dtypes

    bf16 = ml_dtypes.bfloat16
    fp8 = ml_dtypes.float8_e4m3
    x = np.asarray(x, dtype=np.float32)
    W_qkv = np.asarray(W_qkv, dtype=np.float32)
    b_qkv = np.asarray(b_qkv, dtype=np.float32)
    W_proj = np.asarray(W_proj, dtype=np.float32)

    ones2D = np.ones((1, 64), dtype=np.float32).astype(bf16)

    def resid8(a, s):
        a8a = (a * s).astype(fp8)
        a8b = (a * s - a8a.astype(np.float32)).astype(fp8)
        return a8a, a8b

    xabs = []
    for b in range(B):
        xa, xb = resid8(np.ascontiguousarray(x[b].T), XS)
        xabs.append((np.ascontiguousarray(xa), np.ascontiguousarray(xb)))

    # q/k column permutation: new col (t*128 + a*32 + f) <- old col
    # (a*64 + t*32 + f) within the core's 256-column slice
    perm = np.zeros(256, dtype=np.int64)
    for t in range(2):
        for a in range(4):
            for f in range(32):
                perm[t * 128 + a * 32 + f] = a * 64 + t * 32 + f

    in_maps = []
    for c in range(NCORES):
        b, g = divmod(c, GROUPS)
        cs = slice(256 * g, 256 * (g + 1))
        Wq = W_qkv[:, cs][:, perm]
        Wk = W_qkv[:, 1024:2048][:, cs][:, perm]
        Wv = W_qkv[:, 2048:3072][:, cs]
        Wfull = np.concatenate([Wq, Wk, Wv], axis=1)
        Wa, Wb = resid8(Wfull, WS)
        bq = b_qkv[cs.start : cs.stop][perm] * QSCALE
        bk = b_qkv[1024 + cs.start : 1024 + cs.stop][perm] * QSCALE
        biasqk = np.ascontiguousarray(
            np.stack([bq[:128], bq[128:], bk[:128], bk[128:]], axis=1)
        ).astype(np.float32)
        Wp_slice = np.ascontiguousarray(W_proj[cs].astype(bf16))
        in_maps.append(
            {
                "xa": xabs[b][0],
                "xb": xabs[b][1],
                "Wa": np.ascontiguousarray(Wa),
                "Wb": np.ascontiguousarray(Wb),
                "Wp": Wp_slice,
                "biasqk": biasqk,
                "ones2D": ones2D,
            }
        )
    return in_maps


def _make_runner(nc):
    """Build the PJRT executable once (mirrors bass2jax.run_bass_via_pjrt)
    so repeated kernel() calls skip re-tracing/compile-cache lookups."""
    import jax
    from jax.sharding import Mesh, PartitionSpec
    from jax.experimental.shard_map import shard_map

    from concourse.bass2jax import (
        _bass_exec_p,
        install_neuronx_cc_hook,
        partition_id_tensor,
    )

    install_neuronx_cc_hook()
    partition_name = (
        nc.partition_id_tensor.name if nc.partition_id_tensor else None
    )
    in_names, out_names, out_avals, zero_outs = [], [], [], []
    for alloc in nc.m.functions[0].allocations:
        if not isinstance(alloc, mybir.MemoryLocationSet):
            continue
        name = alloc.memorylocations[0].name
        if alloc.kind == "ExternalInput":
            if name != partition_name:
                in_names.append(name)
        elif alloc.kind == "ExternalOutput":
            out_names.append(name)
            shape = tuple(alloc.tensor_shape)
            dtype = mybir.dt.np(alloc.dtype)
            out_avals.append(jax.core.ShapedArray(shape, dtype))
            zero_outs.append(np.zeros(shape, dtype))
    n_params = len(in_names)
    all_in_names = in_names + out_names
    if partition_name is not None:
        all_in_names = all_in_names + [partition_name]

    def _body(*args):
        operands = list(args)
        if partition_name is not None:
            operands.append(partition_id_tensor())
        return tuple(
            _bass_exec_p.bind(
                *operands,
                out_avals=tuple(out_avals),
                in_names=tuple(all_in_names),
                out_names=tuple(out_names),
                lowering_input_output_aliases=(),
                sim_require_finite=True,
                sim_require_nnan=True,
                nc=nc,
            )
        )

    devices = jax.devices()[:NCORES]
    mesh = Mesh(np.asarray(devices), ("core",))
    in_specs = (PartitionSpec("core"),) * (n_params + len(out_names))
    out_specs = (PartitionSpec("core"),) * len(out_names)
    fn = jax.jit(
        shard_map(_body, mesh=mesh, in_specs=in_specs,
                  out_specs=out_specs, check_rep=False),
        keep_unused=True,
    )
    concat_zeros = [
        np.zeros((NCORES * z.shape[0], *z.shape[1:]), z.dtype)
        for z in zero_outs
    ]

    def run(in_maps):
        concat_in = [
            np.concatenate([np.asarray(m[name]) for m in in_maps], axis=0)
            for name in in_names
        ]
        out_arrs = fn(*concat_in, *concat_zeros)
        return [
            {
                name: np.asarray(out_arrs[i]).reshape(
                    NCORES, *out_avals[i].shape
                )[c]
                for i, name in enumerate(out_names)
            }
            for c in range(NCORES)
        ]

    return run


def kernel(x, W_qkv, b_qkv, W_proj, b_proj):
    if "nc" not in _CACHE:
        _CACHE["nc"] = _build()
        try:
            _CACHE["run"] = _make_runner(_CACHE["nc"])
        except Exception:
            _CACHE["run"] = None
    in_maps = _host_prep(x, W_qkv, b_qkv, W_proj, b_proj)
    results = None
    if _CACHE["run"] is not None:
        try:
            results = _CACHE["run"](in_maps)
        except Exception:
            results = None
    if results is None:
        # fallback: the stock path
        results = run_bass_kernel_spmd(
            _CACHE["nc"], in_maps, core_ids=list(range(NCORES))
        ).results
    out = np.zeros((B, N, D), dtype=np.float32)
    bp = np.asarray(b_proj, dtype=np.float32)
    # v-bias contributes bv @ W_proj to every output row (softmax weights
    # sum to 1), so it folds into the projection bias host-side
    bp = bp + np.asarray(b_qkv, np.float32)[2048:] @ np.asarray(
        W_proj, np.float32)
    for b in range(B):
        acc = results[4 * b]["y"].astype(np.float32).copy()
        for g in range(1, GROUPS):
            acc += results[4 * b + g]["y"]
        out[b] = acc + bp
    return out


# revision 65
# speedup vs baseline: 1.2577x; 1.0029x over previous
"""Causal self-attention (b=2, n=2048, d=1024, 16 heads) on 8 NeuronCores.

Sharding: core c handles batch b = c // 4 and head group g = c % 4
(heads 4g..4g+3).  qkv weights column-sharded, proj weights row-sharded
(Megatron); each core emits a partial [2048, 1024] proj output and the
host sums the 4 partials per batch (biases folded in host-side).

Precision plan (tolerance 2e-2; ~1.5e-2 measured on the real inputs):
  qkv/v projections: fp8e4 DoubleRow with residual compensation — x and
      W are split on host into fp8 main + fp8 residual AT THE SAME SCALE
      (x16 / x64), so the three cross terms (aa, ab, ba) accumulate in
      one PSUM group with no extra vector work.  1.5 cyc/row vs 2 for
      bf16, accuracy better than bf16 (1.2e-3 vs 2.4e-3).
  q, k: cast to fp8 (x16) during the qkv copyback; QK^T runs as fp8
      DoubleRow (0.5 cyc/row): per head the 64 contraction dims live as
      [32 partitions x 2 k-tiles].  Scores land x256 in PSUM; the exp
      scale 0.125/256 folds it back.  (~1.46e-2, the error budget.)
  v / et / masks / onT / W_proj: bf16 (1 cyc/row, 2-byte DVE modes).

Scheduling: ONE global software-pipelined stream over all (quarter,
head-pair, k-block) attention tasks.  QK runs AV_LAG slots ahead of AV;
exp/mask sit between on ACT/DVE.  PE fillers — V blocks, the previous
quarter's output projection, the NEXT quarter's qkv m-tiles — are paced
into the stream so PE never waits on the ACT exp feed, and boundary
drains (hp/quarter) are covered by the next segment's QKs.  Softmax
normalization is deferred via a slot-scheduled action queue: reciprocal
(from PSUM row 64) + drain right after a segment's last AV, the PE
ones-broadcast + 2-byte multiply a few slots later.
"""
import sys

sys.path.insert(0, "/opt/trn_rl_repo")

import numpy as np

import concourse.bass as bass  # noqa: F401
import concourse.mybir as mybir
import concourse.tile as tile
from concourse import bacc
from concourse.bass_utils import run_bass_kernel_spmd

F32 = mybir.dt.float32
F32R = mybir.dt.float32r
BF16 = mybir.dt.bfloat16
FP8 = mybir.dt.float8e4
Exp = mybir.ActivationFunctionType.Exp
DR = mybir.MatmulPerfMode.DoubleRow

B = 2
N = 2048
D = 1024
NH = 16
HD = 64
NCORES = 8
GROUPS = 4                # head groups (cores per batch)
HPC = NH // GROUPS        # heads per core = 4
PAIRS = HPC // 2          # head pairs per core = 2
QS = 512                  # q_super width
NQS = N // QS             # 4
NB = N // 128             # 16 token blocks
CCH = D // 128            # 8 contraction chunks
NDR = CCH // 2            # DoubleRow chunk pairs = 4
XS = 16.0                 # fp8 scale for x
WS = 64.0                 # fp8 scale for W_qkv
QSCALE = 16.0             # fp8 scale for q and k
PSCALE = XS * WS          # qkv PSUM arrives x1024
ESCALE = 0.125 / (QSCALE * QSCALE)
AV_LAG = 4

_CACHE = {}


def _build():
    nc = bacc.Bacc("TRN2", target_bir_lowering=False, debug=False,
                   num_devices=NCORES)
    xa_d = nc.dram_tensor("xa", [D, N], FP8, kind="ExternalInput").ap()
    xb_d = nc.dram_tensor("xb", [D, N], FP8, kind="ExternalInput").ap()
    Wa_d = nc.dram_tensor("Wa", [D, 768], FP8, kind="ExternalInput").ap()
    Wb_d = nc.dram_tensor("Wb", [D, 768], FP8, kind="ExternalInput").ap()
    Wp_d = nc.dram_tensor("Wp", [256, D], BF16, kind="ExternalInput").ap()
    biasqk = nc.dram_tensor("biasqk", [128, 4], F32, kind="ExternalInput").ap()
    ones2D = nc.dram_tensor("ones2D", [1, 64], BF16, kind="ExternalInput").ap()
    y = nc.dram_tensor("y", [N, D], BF16, kind="ExternalOutput").ap()

    with tile.TileContext(nc) as tc:
        with (
            tc.tile_pool(name="persist", bufs=1) as pp,
            tc.tile_pool(name="xtq_pool", bufs=3) as xtq_pool,
            tc.tile_pool(name="et_pool", bufs=10) as et_pool,
            tc.tile_pool(name="work", bufs=3) as work,
            tc.tile_pool(name="ysb_pool", bufs=6) as ysb_pool,
            tc.tile_pool(name="mm", bufs=2, space="PSUM") as mm,
            tc.tile_pool(name="spool", bufs=2, space="PSUM") as spool,
            tc.tile_pool(name="opool", bufs=2, space="PSUM") as opool,
        ):
            # ---- persistent tiles ----
            Wa_sb = pp.tile([128, CCH, 768], FP8)
            Wb_sb = pp.tile([128, CCH, 768], FP8)
            Wp_sb = pp.tile([128, 2, D], BF16)
            bqk_sb = pp.tile([128, 4], F32)
            ones2 = pp.tile([1, 64], BF16)
            qT = pp.tile([128, 2, N], FP8)
            kT = pp.tile([128, 2, N], FP8)
            onT = pp.tile([128, PAIRS, N], BF16)
            vaug = pp.tile([128, NB, HPC * 65], BF16)
            vaug_h = vaug.rearrange("p b (h c) -> p b h c", c=65)
            masks = pp.tile([128, 4, QS], BF16)

            Wa_r = Wa_d.rearrange("(c p) f -> p c f", p=128)
            Wb_r = Wb_d.rearrange("(c p) f -> p c f", p=128)
            xa_r = xa_d.rearrange("(c p) n -> p c n", p=128)
            xb_r = xb_d.rearrange("(c p) n -> p c n", p=128)
            Wp_r = Wp_d.rearrange("(c p) f -> p c f", p=128)
            y_r = y.rearrange("(t p) f -> t p f", p=128)

            # causal masks on gpsimd (off the DMA critical path):
            # masks[p, t, q] = 1.0 iff q - p - 128*t >= 0
            nc.gpsimd.memset(masks[:], 1.0)
            for t in range(4):
                nc.gpsimd.affine_select(
                    out=masks[:, t, :],
                    in_=masks[:, t, :],
                    compare_op=mybir.AluOpType.is_ge,
                    fill=0.0,
                    base=-128 * t,
                    pattern=[[1, QS]],
                    channel_multiplier=-1,
                )

            def fetch_xq(q):
                """prefetch a quarter of x (both residual halves) as two
                big DMAs on the gpsimd SWDGE queue (off the qSP/qAct
                critical paths)"""
                t0, t1 = QS * q, QS * (q + 1)
                xqa = xtq_pool.tile([128, CCH, QS], FP8, tag="xa",
                                    name=f"xa{q}")
                xqb = xtq_pool.tile([128, CCH, QS], FP8, tag="xb",
                                    name=f"xb{q}")
                nc.gpsimd.dma_start(xqa[:, :, :], xa_r[:, :, t0:t1])
                nc.gpsimd.dma_start(xqb[:, :, :], xb_r[:, :, t0:t1])
                return (xqa, xqb)

            def dr3(ps, pairs, first, last):
                """residual DoubleRow passes: pairs yields (lhsT, rhs) APs
                ordered so the main (a,a) terms go first — compute can
                start before the residual tensors finish loading"""
                n = len(pairs)
                for pi, (lh, rh) in enumerate(pairs):
                    nc.tensor.matmul(
                        ps, lh, rh,
                        start=(first and pi == 0),
                        stop=(last and pi == n - 1),
                        perf_mode=DR,
                    )

            def make_qkv_units(q, xq):
                """qkv q/k m-tiles for quarter q, one unit per m-tile
                (kept atomic: the PSUM tile's writers/readers must emit
                consecutively for safe pool recycling).  m = 0,1: q
                feature-tiles 0/1; m = 2,3: k tiles (partition 32a+f maps
                head a, feat f / f+32).  Copyback casts to fp8 with x16
                scale (+ prescaled bias)."""
                xqa, xqb = xq
                ts, te = QS * q, QS * (q + 1)

                def unit(m):
                    def emit():
                        ps = mm.tile([128, QS], F32, tag="mm",
                                     name=f"qk{q}{m}")
                        msl = slice(128 * m, 128 * (m + 1))
                        pairs = []
                        for wsb, xsb in ((Wa_sb, xqa), (Wb_sb, xqa),
                                         (Wa_sb, xqb)):
                            for dr in range(NDR):
                                sl = slice(2 * dr, 2 * dr + 2)
                                pairs.append((wsb[:, sl, msl],
                                              xsb[:, sl, :]))
                        dr3(ps[:], pairs, True, True)
                        dst = qT if m < 2 else kT
                        nc.vector.tensor_scalar(
                            out=dst[:, m % 2, ts:te],
                            in0=ps[:],
                            scalar1=QSCALE / PSCALE,
                            scalar2=bqk_sb[:, m : m + 1],
                            op0=mybir.AluOpType.mult,
                            op1=mybir.AluOpType.add,
                        )
                    return emit
                return [unit(m) for m in range(4)]

            def make_v_units(q, xq):
                """V token-major into [V|1] slots (psum x1024 -> /1024)"""
                xqa, xqb = xq

                def unit(blk):
                    def emit():
                        tb = 4 * q + blk
                        vps = mm.tile([128, 256], F32, tag="mm",
                                      name=f"v{q}{blk}")
                        bsl = slice(128 * blk, 128 * (blk + 1))
                        pairs = []
                        for xsb, wsb in ((xqa, Wa_sb), (xqa, Wb_sb),
                                         (xqb, Wa_sb)):
                            for dr in range(NDR):
                                sl = slice(2 * dr, 2 * dr + 2)
                                pairs.append((xsb[:, sl, bsl],
                                              wsb[:, sl, 512:768]))
                        dr3(vps[:], pairs, True, True)
                        nc.vector.tensor_scalar(
                            out=vaug_h[:, tb, :, 0:64],
                            in0=vps.rearrange("p (h c) -> p h c", c=64),
                            scalar1=1.0 / PSCALE,
                            scalar2=None,
                            op0=mybir.AluOpType.mult,
                        )
                    return emit
                return [unit(blk) for blk in range(4)]

            def make_proj_units(jj, tail=False):
                """output projection for quarter jj, one (block, half) unit"""
                def unit(blk, nh):
                    def emit():
                        tb = 4 * jj + blk
                        yps = mm.tile([128, QS], F32, tag="mm",
                                      name=f"y{tb}{nh}")
                        for c in range(2):
                            nc.tensor.matmul(
                                yps[:],
                                onT[:, c, 128 * tb : 128 * (tb + 1)],
                                Wp_sb[:, c, QS * nh : QS * (nh + 1)],
                                start=(c == 0),
                                stop=(c == 1),
                            )
                        ysb = ysb_pool.tile([128, QS], BF16, tag="ysb",
                                            name=f"ysb{tb}{nh}")
                        # tail: ACT is idle — alternate copy engines
                        if tail and (blk + nh) % 2 == 1:
                            nc.scalar.copy(ysb[:], yps[:])
                        else:
                            nc.vector.tensor_copy(ysb[:], yps[:])
                        nc.sync.dma_start(
                            y_r[tb][:, QS * nh : QS * (nh + 1)], ysb[:]
                        )
                    return emit
                return [unit(blk, nh) for blk in range(4) for nh in range(2)]

            # ---- global attention stream state ----
            tasks = [(j, hp, i)
                     for j in range(NQS)
                     for hp in range(PAIRS)
                     for i in range(4 * j + 4)]
            ets = {}
            o_ps_map = {}
            sched = {}           # slot -> [closures]
            fq_v, fq_qkv, fq_proj = [], [], []

            def emit_qk(j, hp, i):
                t = i - 4 * j
                qs0 = 0 if t < 0 else 128 * t
                sps = spool.tile([128, 2, QS], F32, tag="s",
                                 name=f"s{j}{hp}{i}")
                for h in range(2):
                    a = 2 * hp + h
                    nc.tensor.matmul(
                        sps[:, h, qs0:],
                        kT[32 * a : 32 * a + 32, :, 128 * i : 128 * (i + 1)],
                        qT[32 * a : 32 * a + 32, :,
                           QS * j + qs0 : QS * (j + 1)],
                        start=True,
                        stop=True,
                        perf_mode=DR,
                        tile_position=(32 * a, 0),
                    )
                et = et_pool.tile([128, 2, QS], BF16, tag="et",
                                  name=f"et{j}{hp}{i}")
                nc.scalar.activation(
                    et[:, :, qs0:], sps[:, :, qs0:], Exp, scale=ESCALE,
                )
                if t >= 0:
                    mhi = 128 * t + 128
                    nc.vector.tensor_mul(
                        et[:, :, qs0:mhi],
                        et[:, :, qs0:mhi],
                        masks[:, t, qs0:mhi].unsqueeze(1)
                        .broadcast_to([128, 2, mhi - qs0]),
                    )
                ets[(j, hp, i)] = et

            def norm_finish(j, hp, osb, rec2, on_act=False):
                """ones-broadcast + normalize multiply for segment (j, hp);
                one broadcast matmul per head (output column bases 0/64 —
                single-partition writes off base 0 are illegal, so the two
                reciprocals live on the free dim of one 1-partition tile)"""
                bc_ps = mm.tile([128, QS], F32, tag="mm", name=f"bc{j}{hp}")
                for h in range(2):
                    nc.tensor.matmul(bc_ps[64 * h : 64 * h + 64, :],
                                     ones2[:], rec2[:, h, :],
                                     start=True, stop=True)
                bc_sb = work.tile([128, QS], BF16, tag="bc_sb",
                                  name=f"bs{j}{hp}")
                if on_act:
                    nc.scalar.copy(bc_sb[:], bc_ps[:])
                else:
                    nc.vector.tensor_copy(bc_sb[:], bc_ps[:])
                nc.vector.tensor_mul(
                    onT[:, hp, QS * j : QS * (j + 1)], osb[:], bc_sb[:]
                )
                if hp == PAIRS - 1 and j + 1 < NQS:
                    fq_proj.extend(make_proj_units(j))

            # tail staircase state (used by emit_av on the final
            # segment); tiles and proj units are created lazily at first
            # use so the work-pool rings are at their current position
            jt, hpt = NQS - 1, PAIRS - 1
            tail = {}

            def tail_recip(ch):
                if not tail:
                    tail["rec"] = work.tile([1, 2, QS], BF16, tag="rec",
                                            name="rect")
                    tail["osb"] = work.tile([128, QS], BF16, tag="osb",
                                            bufs=4, name="osbt")
                    tail["proj"] = make_proj_units(jt, tail=True)
                o_ps = o_ps_map[(jt, hpt)]
                csl = slice(128 * ch, 128 * (ch + 1))
                for h in range(2):
                    with nc.allow_low_precision("f32r recip"):
                        nc.vector.reciprocal(tail["rec"][:, h, csl],
                                             o_ps[h][64:65, csl])
                    nc.vector.tensor_copy(
                        tail["osb"][64 * h : 64 * h + 64, csl],
                        o_ps[h][0:64, csl])

            def tail_norm_proj(ch):
                csl = slice(128 * ch, 128 * (ch + 1))
                bc_ps = mm.tile([128, 128], F32, tag="mm", name=f"bct{ch}")
                for h in range(2):
                    nc.tensor.matmul(bc_ps[64 * h : 64 * h + 64, :],
                                     ones2[:], tail["rec"][:, h, csl],
                                     start=True, stop=True)
                bc_sb = work.tile([128, 128], BF16, tag="bc_sb",
                                  name=f"bst{ch}")
                if ch % 2 == 0:
                    nc.scalar.copy(bc_sb[:], bc_ps[:])
                else:
                    nc.vector.tensor_copy(bc_sb[:], bc_ps[:])
                nc.vector.tensor_mul(
                    onT[:, hpt, QS * jt + 128 * ch : QS * jt + 128 * (ch + 1)],
                    tail["osb"][:, csl], bc_sb[:],
                )
                # token block 4*jt+ch is now normalized for both pairs
                tail["proj"][2 * ch]()
                tail["proj"][2 * ch + 1]()

            def emit_av(s, j, hp, i):
                t = i - 4 * j
                qs0 = 0 if t < 0 else 128 * t
                n_i = 4 * j + 4
                if hp == 0 and t == 0:
                    # diagonal AVs consume this quarter's V — force-flush
                    while fq_v:
                        fq_v.pop(0)()
                if i == 0:
                    o_ps_map[(j, hp)] = {
                        h: opool.tile([65, QS], F32, tag="o",
                                      name=f"o{j}{hp}{h}")
                        for h in range(2)
                    }
                o_ps = o_ps_map[(j, hp)]
                et = ets.pop((j, hp, i))
                if (j, hp) == (NQS - 1, PAIRS - 1):
                    # final segment: same trimmed matmuls, but with the
                    # group check skipped (bank flags stay cleared) so the
                    # tail staircase can read finalized o columns while
                    # later diagonal blocks still accumulate — block t
                    # only writes columns >= 128t, so chunk t is final
                    # once block n_i-4+t has landed
                    for h in range(2):
                        hh = (2 * hp + h) * 65
                        nc.tensor.matmul(
                            o_ps[h][:, qs0:],
                            vaug[:, i, hh : hh + 65],
                            et[:, h, qs0:],
                            start=(i == 0),
                            stop=(i == n_i - 1),
                            skip_group_check=True,
                        )
                    if t >= 0:
                        # staircase: column chunk t is final — normalize
                        # it (and run the previous chunk's broadcast,
                        # staggered so PE never waits on the reciprocal)
                        tail_recip(t)
                        if t >= 1:
                            tail_norm_proj(t - 1)
                elif False:
                    pass
                else:
                    for h in range(2):
                        hh = (2 * hp + h) * 65
                        nc.tensor.matmul(
                            o_ps[h][:, qs0:],
                            vaug[:, i, hh : hh + 65],
                            et[:, h, qs0:],
                            start=(i == 0),
                            stop=(i == n_i - 1),
                        )
                if i == n_i - 1 and (j, hp) != (NQS - 1, PAIRS - 1):
                    # segment done: reciprocal from PSUM row 64 + numerator
                    # drain now (frees PSUM); broadcast+multiply a few slots
                    # later so PE's bc matmul never waits on the reciprocal
                    rec2 = work.tile([1, 2, QS], BF16, tag="rec",
                                     name=f"rec{j}{hp}")
                    osb = work.tile([128, QS], BF16, tag="osb", bufs=4,
                                    name=f"osb{j}{hp}")
                    for h in range(2):
                        with nc.allow_low_precision("f32r recip"):
                            nc.vector.reciprocal(rec2[:, h, :],
                                                 o_ps[h][64:65, :])
                        nc.vector.tensor_copy(osb[64 * h : 64 * h + 64, :],
                                              o_ps[h][0:64, :])
                    del o_ps_map[(j, hp)]
                    sched.setdefault(s + 3, []).append(
                        lambda: norm_finish(j, hp, osb, rec2))

            # ---- initial DMAs: big consolidated transfers, spread over
            # the three DGE queues (qSP: x main + y out, qAct: weights,
            # gpsimd SWDGE: x residual + prefetches) so descriptor
            # generation never serializes the startup feed ----
            xqa0 = xtq_pool.tile([128, CCH, QS], FP8, tag="xa", name="xa0")
            xqb0 = xtq_pool.tile([128, CCH, QS], FP8, tag="xb", name="xb0")
            # quarter 0's q/k inputs pair-interleaved: the wire is the
            # startup bottleneck, so feed in consumption order
            for dr in range(NDR):
                sl = slice(2 * dr, 2 * dr + 2)
                nc.sync.dma_start(xqa0[:, sl, :], xa_r[:, sl, 0:QS])
                nc.scalar.dma_start(Wa_sb[:, sl, 0:512], Wa_r[:, sl, 0:512])
            nc.gpsimd.dma_start(xqb0[:, :, :], xb_r[:, :, 0:QS])
            nc.scalar.dma_start(Wb_sb[:, :, 0:512], Wb_r[:, :, 0:512])
            nc.sync.dma_start(bqk_sb[:], biasqk)
            nc.sync.dma_start(ones2[:], ones2D)
            # ones columns of [V|1] via DVE (0*x + 1)
            nc.vector.tensor_scalar(
                out=vaug_h[:, :, :, 64],
                in0=Wa_sb[:, 0, 0:64].rearrange("p (a b) -> p a b", b=HPC),
                scalar1=0.0,
                scalar2=1.0,
                op0=mybir.AluOpType.mult,
                op1=mybir.AluOpType.add,
            )
            # v-columns next (consumed by V fillers early in the stream)
            nc.scalar.dma_start(Wa_sb[:, :, 512:768], Wa_r[:, :, 512:768])
            nc.scalar.dma_start(Wb_sb[:, :, 512:768], Wb_r[:, :, 512:768])
            xq_next = fetch_xq(1)
            nc.scalar.dma_start(Wp_sb[:, :, :], Wp_r[:, :, :])

            # quarter 0's qkv runs up front (nothing to overlap with)
            xq_cur = (xqa0, xqb0)
            for u in make_qkv_units(0, xq_cur):
                u()

            # ---- the global stream ----
            q_first_slot = {}
            acc = 0
            for j in range(NQS):
                q_first_slot[j] = acc
                acc += 2 * (4 * j + 4)
            total_slots = acc

            pace = {}

            def run_sched(s):
                for fn in sched.pop(s, ()):
                    fn()

            # proj units have no deadline — defer them toward quarter 3,
            # where the exp feed (ACT) otherwise outpaces PE's work supply
            PROJ_FRAC = {0: 0.0, 1: 0.0, 2: 0.05, 3: 1.0}

            def pop_fillers(s, j):
                start = q_first_slot[j]
                span = 2 * (4 * j + 4)
                frac = (s - start + 1) / span
                # V done by 40% of quarter (diag AVs), qkv by 85% (the
                # copyback must beat the next quarter's first QK)
                for q_, tgt, key in ((fq_v, 0.40, "v"), (fq_qkv, 0.85, "q")):
                    quota = pace[key + "n"]
                    want = int(quota * min(1.0, frac / tgt) + 0.999)
                    while pace[key] < want and q_:
                        q_.pop(0)()
                        pace[key] += 1
                supply = pace["p"] + len(fq_proj)
                want = int(supply * PROJ_FRAC[j] * frac + 0.999)
                if j == NQS - 1:
                    want = min(want, max(0, supply - 4))  # drain reserve
                while pace["p"] < want and fq_proj:
                    fq_proj.pop(0)()
                    pace["p"] += 1

            for s, (j, hp, i) in enumerate(tasks):
                if hp == 0 and i == 0:
                    # quarter boundary: this quarter's qT/kT must be done
                    while fq_qkv:
                        fq_qkv.pop(0)()
                    fq_v.extend(make_v_units(j, xq_cur))
                    if j + 1 < NQS:
                        xq_cur = xq_next
                        fq_qkv.extend(make_qkv_units(j + 1, xq_cur))
                        if j + 2 < NQS:
                            xq_next = fetch_xq(j + 2)
                    pace.update(vn=len(fq_v), qn=len(fq_qkv),
                                v=0, q=0, p=pace.get("p", 0))
                    if j > 0 and fq_v:
                        # one V filler between the qkv flush and the first
                        # QK so DVE can finish the qT/kT copybacks
                        fq_v.pop(0)()
                        pace["v"] += 1
                emit_qk(j, hp, i)
                run_sched(s)
                if s >= AV_LAG:
                    emit_av(s, *tasks[s - AV_LAG])
                pop_fillers(s, j)

            # ---- drain ----
            for idx in range(AV_LAG):
                s = total_slots + idx
                emit_av(s, *tasks[s - AV_LAG])
                run_sched(s)
                while fq_v:
                    fq_v.pop(0)()
                if fq_proj:
                    fq_proj.pop(0)()
            for s in sorted(list(sched)):
                run_sched(s)
            while fq_proj:
                fq_proj.pop(0)()
            tail_norm_proj(3)

    nc.compile()
    return nc


def _host_prep(x, W_qkv, b_qkv, W_proj, b_proj):
    """Build per-core input maps."""
    import ml_dtypes

    bf16 = ml_dtypes.bfloat16
    fp8 = ml_dtypes.float8_e4m3
    x = np.asarray(x, dtype=np.float32)
    W_qkv = np.asarray(W_qkv, dtype=np.float32)
    b_qkv = np.asarray(b_qkv, dtype=np.float32)
    W_proj = np.asarray(W_proj, dtype=np.float32)

    ones2D = np.ones((1, 64), dtype=np.float32).astype(bf16)

    def resid8(a, s):
        a8a = (a * s).astype(fp8)
        a8b = (a * s - a8a.astype(np.float32)).astype(fp8)
        return a8a, a8b

    xabs = []
    for b in range(B):
        xa, xb = resid8(np.ascontiguousarray(x[b].T), XS)
        xabs.append((np.ascontiguousarray(xa), np.ascontiguousarray(xb)))

    # q/k column permutation: new col (t*128 + a*32 + f) <- old col
    # (a*64 + t*32 + f) within the core's 256-column slice
    perm = np.zeros(256, dtype=np.int64)
    for t in range(2):
        for a in range(4):
            for f in range(32):
                perm[t * 128 + a * 32 + f] = a * 64 + t * 32 + f

    in_maps = []
    for c in range(NCORES):
        b, g = divmod(c, GROUPS)
        cs = slice(256 * g, 256 * (g + 1))
        Wq = W_qkv[:, cs][:, perm]
        Wk = W_qkv[:, 1024:2048][:, cs][:, perm]
        Wv = W_qkv[:, 2048:3072][:, cs]
        Wfull = np.concatenate([Wq, Wk, Wv], axis=1)
        Wa, Wb = resid8(Wfull, WS)
        bq = b_qkv[cs.start : cs.stop][perm] * QSCALE
        bk = b_qkv[1024 + cs.start : 1024 + cs.stop][perm] * QSCALE
        biasqk = np.ascontiguousarray(
            np.stack([bq[:128], bq[128:], bk[:128], bk[128:]], axis=1)
        ).astype(np.float32)
        Wp_slice = np.ascontiguousarray(W_proj[cs].astype(bf16))
        in_maps.append(
            {
                "xa": xabs[b][0],
                "xb": xabs[b][1],
                "Wa": np.ascontiguousarray(Wa),
                "Wb": np.ascontiguousarray(Wb),
                "Wp": Wp_slice,
                "biasqk": biasqk,
                "ones2D": ones2D,
            }
        )
    return in_maps


def _make_runner(nc):
    """Build the PJRT executable once (mirrors bass2jax.run_bass_via_pjrt)
    so repeated kernel() calls skip re-tracing/compile-cache lookups."""
    import jax
    from jax.sharding import Mesh, PartitionSpec
    from jax.experimental.shard_map import shard_map

    from concourse.bass2jax import (
        _bass_exec_p,
        install_neuronx_cc_hook,
        partition_id_tensor,
    )

    install_neuronx_cc_hook()
    partition_name = (
        nc.partition_id_tensor.name if nc.partition_id_tensor else None
    )
    in_names, out_names, out_avals, zero_outs = [], [], [], []
    for alloc in nc.m.functions[0].allocations:
        if not isinstance(alloc, mybir.MemoryLocationSet):
            continue
        name = alloc.memorylocations[0].name
        if alloc.kind == "ExternalInput":
            if name != partition_name:
                in_names.append(name)
        elif alloc.kind == "ExternalOutput":
            out_names.append(name)
            shape = tuple(alloc.tensor_shape)
            dtype = mybir.dt.np(alloc.dtype)
            out_avals.append(jax.core.ShapedArray(shape, dtype))
            zero_outs.append(np.zeros(shape, dtype))
    n_params = len(in_names)
    all_in_names = in_names + out_names
    if partition_name is not None:
        all_in_names = all_in_names + [partition_name]

    def _body(*args):
        operands = list(args)
        if partition_name is not None:
            operands.append(partition_id_tensor())
        return tuple(
            _bass_exec_p.bind(
                *operands,
                out_avals=tuple(out_avals),
                in_names=tuple(all_in_names),
                out_names=tuple(out_names),
                lowering_input_output_aliases=(),
                sim_require_finite=True,
                sim_require_nnan=True,
                nc=nc,
            )
        )

    devices = jax.devices()[:NCORES]
    mesh = Mesh(np.asarray(devices), ("core",))
    in_specs = (PartitionSpec("core"),) * (n_params + len(out_names))
    out_specs = (PartitionSpec("core"),) * len(out_names)
    fn = jax.jit(
        shard_map(_body, mesh=mesh, in_specs=in_specs,
                  out_specs=out_specs, check_rep=False),
        keep_unused=True,
    )
    concat_zeros = [
        np.zeros((NCORES * z.shape[0], *z.shape[1:]), z.dtype)
        for z in zero_outs
    ]

    def run(in_maps):
        concat_in = [
            np.concatenate([np.asarray(m[name]) for m in in_maps], axis=0)
            for name in in_names
        ]
        out_arrs = fn(*concat_in, *concat_zeros)
        return [
            {
                name: np.asarray(out_arrs[i]).reshape(
                    NCORES, *out_avals[i].shape
                )[c]
                for i, name in enumerate(out_names)
            }
            for c in range(NCORES)
        ]

    return run


def kernel(x, W_qkv, b_qkv, W_proj, b_proj):
    if "nc" not in _CACHE:
        _CACHE["nc"] = _build()
        try:
            _CACHE["run"] = _make_runner(_CACHE["nc"])
        except Exception:
            _CACHE["run"] = None
    in_maps = _host_prep(x, W_qkv, b_qkv, W_proj, b_proj)
    results = None
    if _CACHE["run"] is not None:
        try:
            results = _CACHE["run"](in_maps)
        except Exception:
            results = None
    if results is None:
        # fallback: the stock path
        results = run_bass_kernel_spmd(
            _CACHE["nc"], in_maps, core_ids=list(range(NCORES))
        ).results
    out = np.zeros((B, N, D), dtype=np.float32)
    bp = np.asarray(b_proj, dtype=np.float32)
    # v-bias contributes bv @ W_proj to every output row (softmax weights
    # sum to 1), so it folds into the projection bias host-side
    bp = bp + np.asarray(b_qkv, np.float32)[2048:] @ np.asarray(
        W_proj, np.float32)
    for b in range(B):
        acc = results[4 * b]["y"].astype(np.float32).copy()
        for g in range(1, GROUPS):
            acc += results[4 * b + g]["y"]
        out[b] = acc + bp
    return out
